# revision 1
# baseline (speedup 1.0000x reference)
"""AdaptiveCTRGCN distributed Trainium2 kernel (8 NeuronCores, batch-parallel).

Shapes (hardcoded): x (32,256,256,25) f32, A (3,25,25), Wq/Wk (4,16,64),
alpha (4,), Wg (4,64,64), gamma/beta (256,).
Per core: 4 samples. Two channel-halves ("tilepairs") of 128 channels
(= 2 groups of 64). BatchNorm statistics all-reduced across the 8 cores.
"""
import sys

sys.path.insert(0, "/opt/trn_rl_repo")

import numpy as np
import ml_dtypes
from concourse import bass, bacc, tile, mybir, bass_utils

F32 = mybir.dt.float32
BF16 = mybir.dt.bfloat16
AF = mybir.ActivationFunctionType
ALU = mybir.AluOpType

N_CORES = 8
B, C, T, V = 32, 256, 256, 25
G, C_g, d_k = 4, 64, 16
BL = B // N_CORES          # samples per core = 4
TP = 2                     # channel halves (128 ch each)
QC = 16                    # 400-col quad-chunk groups per half (16 t each)
QW = 400                   # cols per quad-chunk (16 t * 25 v)
CH = 100                   # cols per matmul chunk (4 t * 25 v)
TV = T * V                 # 6400
N_GLOBAL = float(B * T * V)   # BN sample count per channel
BN_EPS = 1e-5

_CACHE = {}


def _build(single_core=False):
    nc = bacc.Bacc(
        "TRN2", target_bir_lowering=False, debug=False,
        num_devices=1 if single_core else N_CORES,
    )

    x_d = nc.dram_tensor("x", [BL, C, T, V], F32, kind="ExternalInput").ap()
    xw_d = nc.dram_tensor("xw", [TP, 128, 128], BF16, kind="ExternalInput").ap()
    wqk_d = nc.dram_tensor("wqk", [TP, 128, 112], BF16, kind="ExternalInput").ap()
    aphys_d = nc.dram_tensor("aphys", [V, V], F32, kind="ExternalInput").ap()
    talpha_d = nc.dram_tensor("talpha", [V, G], F32, kind="ExternalInput").ap()
    ident_d = nc.dram_tensor("ident", [V, V], F32, kind="ExternalInput").ap()
    sel_d = nc.dram_tensor("sel", [V, 4 * CH], BF16, kind="ExternalInput").ap()
    gb_d = nc.dram_tensor("gb2", [TP, 128, 2], F32, kind="ExternalInput").ap()
    out_d = nc.dram_tensor("out", [BL, C, T, V], F32, kind="ExternalOutput").ap()
    obs_d = nc.dram_tensor("obspill", [BL, TP, 128, TV], BF16,
                           kind="Internal").ap()

    with tile.TileContext(nc) as tc:
        with (
            tc.tile_pool(name="const", bufs=1) as cpool,
            tc.tile_pool(name="xb", bufs=2 * BL) as xbpool,
            tc.tile_pool(name="dram", bufs=2, space="DRAM") as dpool,
        ):
            # ---- constants ----
            xw_sb = []
            wqk_sb = []
            gm_sb = []
            bt_sb = []
            for tp in range(TP):
                t1 = cpool.tile([128, 128], BF16, tag=f"xw{tp}")
                nc.sync.dma_start(t1[:], xw_d[tp])
                xw_sb.append(t1)
                t2 = cpool.tile([128, 112], BF16, tag=f"wqk{tp}")
                nc.sync.dma_start(t2[:], wqk_d[tp])
                wqk_sb.append(t2)
                t3 = cpool.tile([128, 2], F32, tag=f"gb{tp}")
                nc.sync.dma_start(t3[:], gb_d[tp])
                gm_sb.append(t3[:, 0:1])
                bt_sb.append(t3[:, 1:2])
            aphys_sb = cpool.tile([V, V], F32, tag="aphys")
            nc.sync.dma_start(aphys_sb[:], aphys_d[:])
            talpha_sb = cpool.tile([V, G], F32, tag="talpha")
            nc.sync.dma_start(talpha_sb[:], talpha_d[:])
            ident_sb = cpool.tile([V, V], F32, tag="ident")
            nc.sync.dma_start(ident_sb[:], ident_d[:])
            sel_sb = cpool.tile([V, 4 * CH], BF16, tag="sel")
            nc.sync.dma_start(sel_sb[:], sel_d[:])

            # persistent per-(sample, half) tiles
            xb_t = [[xbpool.tile([128, TV], BF16, tag="xb", name="xbt") for _ in range(TP)]
                    for _ in range(BL)]

            # resident ob for the last two (s,tp) blocks
            obr_t = [cpool.tile([128, TV], BF16, tag=f"obr{i}", name=f"obr{i}")
                     for i in range(4)]

            # per-half stat collectors: [sum|ssq] x samples
            stat_c = [cpool.tile([128, 2, BL], F32, tag=f"statc{tp}", name=f"statc{tp}")
                      for tp in range(TP)]

            # ---- phase 1 (scoped pools) ----
            p1 = tc.tile_pool(name="stage", bufs=6)
            stpool = p1.__enter__()
            p1b = tc.tile_pool(name="xwt", bufs=4)
            xwtpool = p1b.__enter__()
            p1c = tc.tile_pool(name="small", bufs=3)
            p1s = tc.tile_pool(name="spill", bufs=4)
            sppool = p1s.__enter__()
            smpool = p1c.__enter__()
            p1d = tc.tile_pool(name="i4a", bufs=4)
            i4apool = p1d.__enter__()
            p1e = tc.tile_pool(name="mm1", bufs=3, space="PSUM")
            mm1pool = p1e.__enter__()
            p1f = tc.tile_pool(name="mm2", bufs=2, space="PSUM")
            mm2pool = p1f.__enter__()
            p1g = tc.tile_pool(name="qkp", bufs=1, space="PSUM")
            qkpool = p1g.__enter__()
            p1h = tc.tile_pool(name="smp", bufs=1, space="PSUM")
            smppool = p1h.__enter__()
            p1i = tc.tile_pool(name="bnp", bufs=1, space="PSUM")
            bnppool = p1i.__enter__()
            for s in range(BL):
                for tp in range(TP):
                    xb = xb_t[s][tp]
                    c0 = 128 * tp

                    # load + cast (32-t chunks, two DMA rings)
                    for hi in range(8):
                        xs = stpool.tile([128, 32, V], F32, tag="stage")
                        eng = nc.sync if hi % 2 == 0 else nc.gpsimd
                        eng.dma_start(
                            xs[:], x_d[s, c0:c0 + 128, 32 * hi:32 * hi + 32, :]
                        )
                        nc.scalar.activation(
                            xb[:, 800 * hi:800 * hi + 800],
                            xs[:].rearrange("p a b -> p (a b)"),
                            AF.Copy,
                        )

                    # qk pass: accumulate over quad-chunks
                    qk_ps = qkpool.tile([112, QW], F32)
                    wqk_ap = wqk_sb[tp][:]
                    qkp_ap = qk_ps[:]
                    for qi in range(QC):
                        nc.tensor.matmul(
                            qkp_ap,
                            wqk_ap,
                            xb[:, QW * qi:QW * qi + QW],
                            start=(qi == 0),
                            stop=(qi == QC - 1),
                        )
                    # reduce over t16 -> q/k per group, tiles at base 0
                    qg = []
                    kg = []
                    for gi in range(2):
                        qt = smpool.tile([16, V], F32, tag=f"qg{gi}", bufs=1)
                        nc.vector.tensor_reduce(
                            qt[:],
                            qk_ps[64 * gi:64 * gi + 16, :].rearrange(
                                "p (t v) -> p v t", t=16, v=V
                            ),
                            axis=mybir.AxisListType.X,
                            op=ALU.add,
                        )
                        qg.append(qt)
                        kt = smpool.tile([16, V], F32, tag=f"kg{gi}", bufs=1)
                        nc.vector.tensor_reduce(
                            kt[:],
                            qk_ps[64 * gi + 32:64 * gi + 48, :].rearrange(
                                "p (t v) -> p v t", t=16, v=V
                            ),
                            axis=mybir.AxisListType.X,
                            op=ALU.add,
                        )
                        kg.append(kt)

                    # pre-compute first LEAD m1 groups (independent of
                    # the adjacency chain) so PE has work during softmax
                    LEAD = 3
                    xwt_q = {}

                    def do_m1(qi):
                        mp = mm1pool.tile([CH, 512], F32, name="mp")
                        for j in range(4):
                            nc.tensor.matmul(
                                mp[:, 128 * j:128 * j + 128],
                                xb[:, QW * qi + CH * j:QW * qi + CH * j + CH],
                                xw_sb[tp][:],
                                start=True,
                                stop=True,
                            )
                        xwt = xwtpool.tile([CH, 512], BF16, tag="xwt",
                                           name="xwt")
                        if qi % 2 == 0:
                            nc.vector.tensor_copy(xwt[:], mp[:])
                        else:
                            nc.scalar.activation(xwt[:], mp[:], AF.Copy)
                        xwt_q[qi] = xwt

                    for jj in range(LEAD):
                        do_m1(jj)

                    # adaptive adjacency per group
                    i4a_t = []
                    for gi in range(2):
                        g = 2 * tp + gi
                        sc_ps = smppool.tile([V, V], F32, tag="smt", name="scps")
                        nc.tensor.matmul(
                            sc_ps[:], qg[gi][:], kg[gi][:], start=True, stop=True
                        )
                        mx = smpool.tile([V, 1], F32, tag="mx")
                        nc.vector.tensor_reduce(
                            mx[:], sc_ps[:], axis=mybir.AxisListType.X,
                            op=ALU.max, negate=True,
                        )
                        nc.scalar.activation(
                            sc_ps[:], sc_ps[:], AF.Exp, bias=mx[:]
                        )
                        smrr = smpool.tile([V, 3], F32, tag="smrr", bufs=1)
                        nc.vector.tensor_reduce(
                            smrr[:, 0:1], sc_ps[:], axis=mybir.AxisListType.X,
                            op=ALU.add
                        )
                        nc.vector.reciprocal(smrr[:, 1:2], smrr[:, 0:1])
                        rst = smrr[:, 2:3]
                        nc.vector.tensor_scalar_mul(
                            rst, smrr[:, 1:2], talpha_sb[:, g:g + 1]
                        )
                        ag = smpool.tile([V, V], F32, tag="ag", bufs=1)
                        nc.vector.scalar_tensor_tensor(
                            ag[:], sc_ps[:], rst, aphys_sb[:],
                            op0=ALU.mult, op1=ALU.add,
                        )
                        agt_ps = smppool.tile([V, V], F32, tag="smt", name="agtps")
                        nc.tensor.transpose(agt_ps[:], ag[:], ident_sb[:])
                        agtb = smpool.tile([V, V], BF16, tag="agtb", bufs=1)
                        nc.scalar.activation(agtb[:], agt_ps[:], AF.Copy)
                        i4a_ps = smppool.tile([CH, CH], F32, tag="smt", name="i4aps")
                        for d in range(4):
                            nc.tensor.matmul(
                                i4a_ps[:, V * d:V * d + V],
                                sel_sb[:, CH * d:CH * d + CH],
                                agtb[:],
                                start=True,
                                stop=True,
                            )
                        i4a = i4apool.tile([CH, CH], BF16, tag="i4a")
                        nc.scalar.activation(i4a[:], i4a_ps[:], AF.Copy)
                        i4a_t.append(i4a)

                    # main pipeline over quad-chunks
                    bnc = bnppool.tile([128, QC, 6], F32, tag="bnc", bufs=1)
                    for jj in range(LEAD, QC + LEAD):
                        if jj < QC:
                            do_m1(jj)
                        if jj >= LEAD:
                            qi = jj - LEAD
                            xwt = xwt_q.pop(qi)
                            op = mm2pool.tile([128, QW], F32)
                            for h in range(4):
                                for gi in range(2):
                                    nc.tensor.matmul(
                                        op[64 * gi:64 * gi + 64,
                                           CH * h:CH * h + CH],
                                        xwt[:, 128 * h + 64 * gi:
                                            128 * h + 64 * gi + 64],
                                        i4a_t[gi][:],
                                        start=True,
                                        stop=True,
                                    )
                            blk = 2 * s + tp
                            if blk >= 4:
                                nc.scalar.activation(
                                    obr_t[blk - 4][:, QW * qi:QW * qi + QW],
                                    op[:], AF.Copy
                                )
                            else:
                                if qi % 4 == 0:
                                    obsp = sppool.tile(
                                        [128, 4 * QW], BF16, tag="spill")
                                nc.scalar.activation(
                                    obsp[:, QW * (qi % 4):QW * (qi % 4) + QW],
                                    op[:], AF.Copy
                                )
                                if qi % 4 == 3:
                                    seng = (nc.sync if (qi // 4) % 2 == 0
                                            else nc.gpsimd)
                                    seng.dma_start(
                                        obs_d[s, tp, :,
                                              QW * (qi - 3):
                                              QW * (qi - 3) + 4 * QW],
                                        obsp[:],
                                    )
                            nc.vector.bn_stats(bnc[:, qi, :], op[:])

                    # aggregate this (s, tp): mean/var -> sum/ssq columns
                    msv = smpool.tile([128, 4], F32, tag="msv", bufs=1)
                    nc.vector.bn_aggr(
                        msv[:, 0:2], bnc[:].rearrange("p a b -> p (a b)")
                    )
                    # sum = mean * TV ; ssq = (var + mean^2) * TV
                    nc.vector.tensor_scalar_mul(
                        stat_c[tp][:, 0, s:s + 1], msv[:, 0:1], float(TV)
                    )
                    m2 = msv[:, 2:3]
                    nc.vector.tensor_tensor(
                        m2, msv[:, 0:1], msv[:, 0:1], op=ALU.mult
                    )
                    nc.vector.tensor_tensor(
                        m2, m2, msv[:, 1:2], op=ALU.add
                    )
                    nc.vector.tensor_scalar_mul(
                        stat_c[tp][:, 1, s:s + 1], m2, float(TV)
                    )

            for pc in (p1i, p1h, p1g, p1f, p1e, p1d, p1c, p1s, p1b):
                pc.__exit__(None, None, None)

            # phase-2 pools
            p2s = tc.tile_pool(name="ys", bufs=14)
            yspool = p2s.__enter__()
            p2o = tc.tile_pool(name="obin", bufs=4)
            obinpool = p2o.__enter__()
            p2sm = tc.tile_pool(name="small2", bufs=2)
            smpool = p2sm.__enter__()

            # ---- all-reduce BN stats ----
            lg = cpool.tile([128, 8], F32, tag="lg")
            loc = lg[:, 0:4]
            for tp in range(TP):
                nc.vector.tensor_reduce(
                    loc[tp:tp + 1] if False else lg[:, tp:tp + 1],
                    stat_c[tp][:, 0, :],
                    axis=mybir.AxisListType.X,
                    op=ALU.add,
                )
                nc.vector.tensor_reduce(
                    lg[:, 2 + tp:3 + tp],
                    stat_c[tp][:, 1, :],
                    axis=mybir.AxisListType.X,
                    op=ALU.add,
                )
            cin = dpool.tile([128, 4], F32)
            cout = dpool.tile([128, 4], F32)
            nc.sync.dma_start(cin[:], lg[:, 0:4])
            if single_core:
                nc.sync.dma_start(cout[:], cin[:])
            else:
                nc.gpsimd.collective_compute(
                    "AllReduce",
                    ALU.add,
                    replica_groups=[list(range(N_CORES))],
                    ins=[cin[:].opt()],
                    outs=[cout[:].opt()],
                )
            glob = lg[:, 4:8]
            nc.sync.dma_start(glob, cout[:])

            # inv = gamma * rsqrt(var+eps); b2 = beta - mu*inv  (per half)
            inv_sb = []
            b2_sb = []
            ivb2 = cpool.tile([128, 4], F32, tag="ivb2")
            for tp in range(TP):
                scr = smpool.tile([128, 6], F32, tag="scr", bufs=1)
                mu = scr[:, 0:1]
                nc.vector.tensor_scalar_mul(
                    mu, lg[:, 4 + tp:5 + tp], 1.0 / N_GLOBAL
                )
                ex2 = scr[:, 1:2]
                nc.vector.tensor_scalar_mul(
                    ex2, lg[:, 6 + tp:7 + tp], 1.0 / N_GLOBAL
                )
                mu2 = scr[:, 2:3]
                nc.vector.tensor_tensor(mu2, mu, mu, op=ALU.mult)
                var = scr[:, 3:4]
                nc.vector.tensor_tensor(var, ex2, mu2, op=ALU.subtract)
                nc.vector.tensor_scalar_add(var, var, BN_EPS)
                sq = scr[:, 4:5]
                nc.scalar.activation(sq, var, AF.Sqrt)
                rs = scr[:, 5:6]
                nc.vector.reciprocal(rs, sq)
                iv = ivb2[:, tp:tp + 1]
                nc.vector.tensor_tensor(iv, rs, gm_sb[tp], op=ALU.mult)
                inv_sb.append(iv)
                mi = scr[:, 2:3]
                nc.vector.tensor_tensor(mi, mu, iv, op=ALU.mult)
                b2 = ivb2[:, 2 + tp:3 + tp]
                nc.vector.tensor_tensor(b2, bt_sb[tp], mi, op=ALU.subtract)
                b2_sb.append(b2)

            # ---- phase 2: y = x + ob*inv + b2 ----
            for s in range(BL):
                for tp in range(TP):
                    xb = xb_t[s][tp]
                    c0 = 128 * tp
                    blk = 2 * s + tp
                    for ci in range(4):
                        if blk >= 4:
                            obin = obr_t[blk - 4][:, 4 * QW * ci:
                                                  4 * QW * ci + 4 * QW]
                        else:
                            obt = obinpool.tile([128, 4 * QW], BF16, tag="obin")
                            ieng = nc.sync if ci % 2 == 0 else nc.gpsimd
                            ieng.dma_start(
                                obt[:],
                                obs_d[s, tp, :,
                                      4 * QW * ci:4 * QW * ci + 4 * QW],
                            )
                            obin = obt[:]
                        for h in range(4):
                            lo = 400 * h
                            sl = slice(4 * QW * ci + lo, 4 * QW * ci + lo + 400)
                            ys = yspool.tile([128, 16, V], F32, tag="ys")
                            ysf = ys[:].rearrange("p a b -> p (a b)")
                            nc.scalar.activation(
                                ysf, obin[:, lo:lo + 400], AF.Identity,
                                scale=inv_sb[tp], bias=b2_sb[tp],
                            )
                            nc.vector.tensor_tensor(
                                ysf, ysf, xb[:, sl], op=ALU.add,
                            )
                            eng2 = nc.sync if h % 2 == 0 else nc.gpsimd
                            t0 = 16 * (4 * ci + h)
                            eng2.dma_start(
                                out_d[s, c0:c0 + 128, t0:t0 + 16, :], ys[:]
                            )
            p2sm.__exit__(None, None, None)
            p2o.__exit__(None, None, None)
            p2s.__exit__(None, None, None)
            p1.__exit__(None, None, None)

    nc.compile()
    return nc


def _host_prep(x, A, Wq, Wk, alpha, Wg, gamma, beta):
    bf = ml_dtypes.bfloat16
    A_sum = A.sum(axis=0)
    A_phys = A_sum / np.clip(A_sum.sum(axis=-1, keepdims=True), 1e-6, None)
    scl = 1.0 / (T * d_k ** 0.25)

    xw = np.zeros((TP, 128, 128), np.float32)
    wqk = np.zeros((TP, 128, 112), np.float32)
    for tp in range(TP):
        for gi in range(2):
            g = 2 * tp + gi
            r = slice(64 * gi, 64 * gi + 64)
            xw[tp][r, r] = Wg[g].T
            wqk[tp][r, 64 * gi:64 * gi + 16] = scl * Wq[g].T
            wqk[tp][r, 64 * gi + 32:64 * gi + 48] = scl * Wk[g].T

    talpha = np.repeat(np.tanh(alpha)[None, :], V, axis=0).astype(np.float32)
    sel = np.zeros((V, 4 * CH), np.float32)
    for d in range(4):
        sel[:, CH * d + V * d:CH * d + V * d + V] = np.eye(V)
    common = {
        "sel": sel.astype(bf),
        "xw": xw.astype(bf),
        "wqk": wqk.astype(bf),
        "aphys": A_phys.astype(np.float32),
        "talpha": talpha,
        "ident": np.eye(V, dtype=np.float32),
        "gb2": np.stack([gamma.reshape(TP, 128), beta.reshape(TP, 128)],
                        axis=-1).astype(np.float32),
    }
    return common


def kernel(x, A, Wq, Wk, alpha, Wg, gamma, beta, _trace=False, _trace_kwargs=None):
    x = np.asarray(x, np.float32)
    common = _host_prep(
        x,
        np.asarray(A, np.float32),
        np.asarray(Wq, np.float32),
        np.asarray(Wk, np.float32),
        np.asarray(alpha, np.float32),
        np.asarray(Wg, np.float32),
        np.asarray(gamma, np.float32),
        np.asarray(beta, np.float32),
    )
    if "nc" not in _CACHE:
        _CACHE["nc"] = _build()
    nc = _CACHE["nc"]

    in_maps = []
    for ci in range(N_CORES):
        m = dict(common)
        m["x"] = np.ascontiguousarray(x[BL * ci:BL * ci + BL])
        in_maps.append(m)

    kw = {}
    if _trace:
        kw = dict(trace=True, trace_kwargs=_trace_kwargs or {})
    res = bass_utils.run_bass_kernel_spmd(
        nc, in_maps, core_ids=list(range(N_CORES)), **kw
    )
    out = np.concatenate([r["out"] for r in res.results], axis=0)
    _CACHE["last_result"] = res
    return out



# revision 40
# speedup vs baseline: 1.5588x; 1.5588x over previous
"""AdaptiveCTRGCN distributed Trainium2 kernel (8 NeuronCores, batch-parallel).

Shapes (hardcoded): x (32,256,256,25) f32, A (3,25,25), Wq/Wk (4,16,64),
alpha (4,), Wg (4,64,64), gamma/beta (256,).
Per core: 4 samples. Two channel-halves ("tilepairs") of 128 channels
(= 2 groups of 64). BatchNorm statistics all-reduced across the 8 cores.

v4: gpsimd casting DMAs move x f32->bf16 straight into resident SBUF tiles.
Phase 1 also copies x -> out via DRAM->DRAM DMA during idle DMA time; the
conv output ob overwrites the input tile in place for every block. Phase 2
computes z = ob*inv+b2 in DVE 4x bf16 mode and DMA-accumulates z into the
output (out += z), so no spill, no re-read, and no residual-add pass.
The adjacency chain of block b+1 is software-pipelined into block b's
matmul loop.
"""
import sys

sys.path.insert(0, "/opt/trn_rl_repo")

import numpy as np
import ml_dtypes
from concourse import bass, bacc, tile, mybir, bass_utils

F32 = mybir.dt.float32
BF16 = mybir.dt.bfloat16
AF = mybir.ActivationFunctionType
ALU = mybir.AluOpType

N_CORES = 8
B, C, T, V = 32, 256, 256, 25
G, C_g, d_k = 4, 64, 16
BL = B // N_CORES          # samples per core = 4
TP = 2                     # channel halves (128 ch each)
NB = 2 * BL                # blocks per core = 8
QC = 16                    # 400-col quad-chunk groups per half (16 t each)
QW = 400                   # cols per quad-chunk (16 t * 25 v)
CH = 100                   # cols per matmul chunk (4 t * 25 v)
PW = 800                   # cols per m1 pair (2 quad-chunks)
TV = T * V                 # 6400
N_GLOBAL = float(B * T * V)   # BN sample count per channel
BN_EPS = 1e-5

# engine for the PSUM->SBUF ob copy per quad-chunk ('a'=ACT, 'd'=DVE)
OB_ENG = ['a', 'd', 'a', 'd', 'a', 'd', 'a', 'd',
          'a', 'd', 'a', 'd', 'a', 'a', 'a', 'a']

_CACHE = {}


def _build(single_core=False):
    nc = bacc.Bacc(
        "TRN2", target_bir_lowering=False, debug=False,
        num_devices=1 if single_core else N_CORES,
    )

    x_d = nc.dram_tensor("x", [BL, C, T, V], F32, kind="ExternalInput").ap()
    xw_d = nc.dram_tensor("xw", [TP, 128, 128], BF16, kind="ExternalInput").ap()
    wqk_d = nc.dram_tensor("wqk", [TP, 128, 112], BF16, kind="ExternalInput").ap()
    aphys_d = nc.dram_tensor("aphys", [V, V], F32, kind="ExternalInput").ap()
    talpha_d = nc.dram_tensor("talpha", [V, G], F32, kind="ExternalInput").ap()
    ident_d = nc.dram_tensor("ident", [V, V], F32, kind="ExternalInput").ap()
    sel_d = nc.dram_tensor("sel", [V, 4 * CH], BF16, kind="ExternalInput").ap()
    gb_d = nc.dram_tensor("gb4", [128, 4], F32, kind="ExternalInput").ap()
    out_d = nc.dram_tensor("out", [BL, C, T, V], F32, kind="ExternalOutput").ap()

    with tile.TileContext(nc) as tc:
        with (
            tc.tile_pool(name="const", bufs=1) as cpool,
            tc.tile_pool(name="dram", bufs=2, space="DRAM") as dpool,
        ):
            # ---- constants ----
            xw_sb = []
            wqk_sb = []
            for tp in range(TP):
                t1 = cpool.tile([128, 128], BF16, tag=f"xw{tp}")
                nc.sync.dma_start(t1[:], xw_d[tp])
                xw_sb.append(t1)
                t2 = cpool.tile([128, 112], BF16, tag=f"wqk{tp}")
                nc.sync.dma_start(t2[:], wqk_d[tp])
                wqk_sb.append(t2)
            gb_sb = cpool.tile([128, 4], F32, tag="gb4")
            nc.sync.dma_start(gb_sb[:], gb_d[:])
            aphys_sb = cpool.tile([V, V], F32, tag="aphys")
            nc.sync.dma_start(aphys_sb[:], aphys_d[:])
            talpha_sb = cpool.tile([V, G], F32, tag="talpha")
            nc.sync.dma_start(talpha_sb[:], talpha_d[:])
            ident_sb = cpool.tile([V, V], F32, tag="ident")
            nc.sync.dma_start(ident_sb[:], ident_d[:])
            sel_sb = cpool.tile([V, 4 * CH], BF16, tag="sel")
            nc.sync.dma_start(sel_sb[:], sel_d[:])

            # persistent per-block tiles (bf16): input copies, overwritten
            # in place by the conv output ob during the block's main loop.
            xb_t = [cpool.tile([128, TV], BF16, tag=f"xb{i}", name=f"xb{i}")
                    for i in range(NB)]
            stat_c = [cpool.tile([128, 2, BL], F32, tag=f"statc{tp}",
                                 name=f"statc{tp}")
                      for tp in range(TP)]
            lg = cpool.tile([128, 8], F32, tag="lg")
            ivb2 = cpool.tile([128, 4], F32, tag="ivb2")

            # ---- phase 1 pools ----
            p1b = tc.tile_pool(name="xwt", bufs=8)
            xwtpool = p1b.__enter__()
            p1c = tc.tile_pool(name="small", bufs=2)
            smpool = p1c.__enter__()
            p1cc = tc.tile_pool(name="bnc", bufs=2)
            bncpool = p1cc.__enter__()
            p1e = tc.tile_pool(name="mm1", bufs=2, space="PSUM")
            mm1pool = p1e.__enter__()
            p1f = tc.tile_pool(name="mm2", bufs=2, space="PSUM")
            mm2pool = p1f.__enter__()
            p1g = tc.tile_pool(name="qkp", bufs=1, space="PSUM")
            qkpool = p1g.__enter__()
            p1h = tc.tile_pool(name="smp", bufs=1, space="PSUM")
            smppool = p1h.__enter__()

            # all input loads up front: casting DMA f32->bf16 on gpsimd
            # (block 0 in 4 chunks so its qk pass starts early)
            for blk in range(NB):
                s, tp = blk // 2, blk % 2
                nch = 4 if blk == 0 else 1
                tpc = T // nch
                for hi in range(nch):
                    nc.gpsimd.dma_start(
                        xb_t[blk][:, TV // nch * hi:TV // nch * (hi + 1)]
                        .rearrange("p (a b) -> p a b", a=tpc, b=V),
                        x_d[s, 128 * tp:128 * tp + 128,
                            tpc * hi:tpc * (hi + 1), :],
                    )
            # residual: out = x, streamed DRAM->DRAM behind the input loads
            # on the same gpsimd queue (phase 2 accumulates ob*inv+b2 on top).
            # Emitted per channel-half so each half's accum stream can start
            # as soon as its own residual copies are done.
            def emit_xout(blk):
                s, tp = blk // 2, blk % 2
                nc.gpsimd.dma_start(
                    out_d[s, 128 * tp:128 * tp + 128, :, :],
                    x_d[s, 128 * tp:128 * tp + 128, :, :],
                )

            for blk in (0, 2, 4, 6):
                emit_xout(blk)

            # per-block pipelined state
            st = {}

            def emit_qk(blk, part):
                # 4 accumulating matmuls per call (part 0..3)
                if part == 0:
                    st.setdefault(blk, {})["qk"] = qkpool.tile(
                        [112, QW], F32, name="qkps")
                qk_ps = st[blk]["qk"]
                tp = blk % 2
                for qi in range(4 * part, 4 * part + 4):
                    nc.tensor.matmul(
                        qk_ps[:],
                        wqk_sb[tp][:],
                        xb_t[blk][:, QW * qi:QW * qi + QW],
                        start=(qi == 0),
                        stop=(qi == QC - 1),
                    )

            def emit_qkred(blk):
                qk_ps = st[blk]["qk"]
                for gi in range(2):
                    qt = smpool.tile([16, V], F32, tag=f"qg{gi}")
                    nc.vector.tensor_reduce(
                        qt[:],
                        qk_ps[64 * gi:64 * gi + 16, :].rearrange(
                            "p (t v) -> p v t", t=16, v=V
                        ),
                        axis=mybir.AxisListType.X,
                        op=ALU.add,
                    )
                    st[blk][f"qg{gi}"] = qt
                    kt = smpool.tile([16, V], F32, tag=f"kg{gi}")
                    nc.vector.tensor_reduce(
                        kt[:],
                        qk_ps[64 * gi + 32:64 * gi + 48, :].rearrange(
                            "p (t v) -> p v t", t=16, v=V
                        ),
                        axis=mybir.AxisListType.X,
                        op=ALU.add,
                    )
                    st[blk][f"kg{gi}"] = kt

            def emit_sc(blk, gi):
                # scores matmul + softmax chain -> ag (f32 [V,V])
                tp = blk % 2
                g = 2 * tp + gi
                sc_ps = smppool.tile([V, V], F32, tag="smt", name="scps")
                nc.tensor.matmul(
                    sc_ps[:],
                    st[blk][f"qg{gi}"][:],
                    st[blk][f"kg{gi}"][:],
                    start=True, stop=True,
                )
                mx = smpool.tile([V, 1], F32, tag=f"mx{gi}")
                nc.vector.tensor_reduce(
                    mx[:], sc_ps[:], axis=mybir.AxisListType.X,
                    op=ALU.max, negate=True,
                )
                nc.scalar.activation(sc_ps[:], sc_ps[:], AF.Exp, bias=mx[:])
                smrr = smpool.tile([V, 3], F32, tag=f"smrr{gi}")
                nc.vector.tensor_reduce(
                    smrr[:, 0:1], sc_ps[:], axis=mybir.AxisListType.X,
                    op=ALU.add
                )
                nc.vector.reciprocal(smrr[:, 1:2], smrr[:, 0:1])
                rst = smrr[:, 2:3]
                nc.vector.tensor_scalar_mul(
                    rst, smrr[:, 1:2], talpha_sb[:, g:g + 1]
                )
                ag = smpool.tile([V, V], F32, tag=f"ag{gi}")
                nc.vector.scalar_tensor_tensor(
                    ag[:], sc_ps[:], rst, aphys_sb[:],
                    op0=ALU.mult, op1=ALU.add,
                )
                st[blk][f"ag{gi}"] = ag

            def emit_tr(blk, gi):
                agt_ps = smppool.tile([V, V], F32, tag="smt", name="agtps")
                nc.tensor.transpose(agt_ps[:], st[blk][f"ag{gi}"][:], ident_sb[:])
                agtb = smpool.tile([V, V], BF16, tag=f"agtb{gi}")
                nc.vector.tensor_copy(agtb[:], agt_ps[:])
                st[blk][f"agtb{gi}"] = agtb

            def emit_i4(blk, gi):
                i4a_ps = smppool.tile([CH, CH], F32, tag="smt", name="i4aps")
                agtb = st[blk][f"agtb{gi}"]
                for d in range(4):
                    nc.tensor.matmul(
                        i4a_ps[:, V * d:V * d + V],
                        sel_sb[:, CH * d:CH * d + CH],
                        agtb[:],
                        start=True, stop=True,
                    )
                i4a = smpool.tile([CH, CH], BF16, tag=f"i4a{gi}")
                nc.scalar.activation(i4a[:], i4a_ps[:], AF.Copy)
                st[blk][f"i4a{gi}"] = i4a

            def emit_pre(blk, m1p=None):
                # standalone prologue for block 0: interleave its own m1
                # pairs so the PE has work while the softmax chain runs
                for part in range(4):
                    emit_qk(blk, part)
                emit_qkred(blk)
                steps = [
                    ("sc", 0), ("m1", 0), ("sc", 1), ("m1", 1),
                    ("tr", 0), ("m1", 2), ("tr", 1), ("m1", 3),
                    ("i4", 0), ("m1", 4), ("i4", 1), ("m1", 5),
                ]
                for kind, arg in steps:
                    if kind == "sc":
                        emit_sc(blk, arg)
                    elif kind == "tr":
                        emit_tr(blk, arg)
                    elif kind == "i4":
                        emit_i4(blk, arg)
                    elif m1p is not None:
                        m1p(arg)

            def make_m1p(blk):
                tp = blk % 2
                xb = xb_t[blk]
                xwt_p = st.setdefault(blk, {}).setdefault("xwt", {})

                def do_m1p(pi):
                    if pi in xwt_p:
                        return
                    mp = mm1pool.tile([CH, 1024], F32, name="mp")
                    for j in range(8):
                        nc.tensor.matmul(
                            mp[:, 128 * j:128 * j + 128],
                            xb[:, PW * pi + CH * j:PW * pi + CH * j + CH],
                            xw_sb[tp][:],
                            start=True, stop=True,
                        )
                    xwt = xwtpool.tile([CH, 1024], BF16, tag="xwt", name="xwt")
                    nc.scalar.activation(xwt[:], mp[:], AF.Copy)
                    xwt_p[pi] = xwt

                return do_m1p

            def emit_main(blk, nxt):
                """m1/m2/copies/bn for blk; interleaves PRE of nxt."""
                tp = blk % 2
                xb = xb_t[blk]
                ob = xb_t[blk]
                do_m1p = make_m1p(blk)
                xwt_p = st[blk]["xwt"]

                do_m1p(0)
                do_m1p(1)
                bnc = bncpool.tile([128, QC, 6], F32, tag="bnc")
                i4a = [st[blk]["i4a0"], st[blk]["i4a1"]]
                for qi in range(QC):
                    pi = qi // 2
                    if qi % 2 == 0 and pi + 2 <= 7:
                        do_m1p(pi + 2)
                    xwt = xwt_p[pi]
                    base = 512 * (qi % 2)
                    op = mm2pool.tile([128, QW], F32)
                    for h in range(4):
                        for gi in range(2):
                            nc.tensor.matmul(
                                op[64 * gi:64 * gi + 64, CH * h:CH * h + CH],
                                xwt[:, base + 128 * h + 64 * gi:
                                    base + 128 * h + 64 * gi + 64],
                                i4a[gi][:],
                                start=True, stop=True,
                            )
                    dst = ob[:, QW * qi:QW * qi + QW]
                    if OB_ENG[qi] == 'd':
                        nc.vector.tensor_copy(dst, op[:])
                    else:
                        nc.scalar.activation(dst, op[:], AF.Copy)
                    lag = 0 if blk == NB - 1 else 3
                    if qi >= lag:
                        qj = qi - lag
                        nc.vector.bn_stats(
                            bnc[:, qj, :], ob[:, QW * qj:QW * qj + QW]
                        )
                    # interleave next block's qk/adjacency chain
                    if nxt is not None:
                        if qi < 4:
                            emit_qk(nxt, qi)
                        elif qi == 4:
                            emit_qkred(nxt)
                        elif qi == 5:
                            emit_sc(nxt, 0)
                        elif qi == 6:
                            emit_sc(nxt, 1)
                        elif qi == 9:
                            emit_tr(nxt, 0)
                        elif qi == 10:
                            emit_tr(nxt, 1)
                        elif qi == 12:
                            emit_i4(nxt, 0)
                        elif qi == 13:
                            emit_i4(nxt, 1)
                if blk != NB - 1:
                    for qj in (QC - 2, QC - 1):
                        nc.vector.bn_stats(
                            bnc[:, qj, :], ob[:, QW * qj:QW * qj + QW]
                        )

                # block stats -> sum/ssq columns
                s = blk // 2
                msv = smpool.tile([128, 4], F32, tag="msv")
                nc.vector.bn_aggr(
                    msv[:, 0:2], bnc[:].rearrange("p a b -> p (a b)")
                )
                nc.vector.tensor_scalar_mul(
                    stat_c[tp][:, 0, s:s + 1], msv[:, 0:1], float(TV)
                )
                m2c = msv[:, 2:3]
                nc.vector.tensor_tensor(
                    m2c, msv[:, 0:1], msv[:, 0:1], op=ALU.mult
                )
                nc.vector.tensor_tensor(m2c, m2c, msv[:, 1:2], op=ALU.add)
                nc.vector.tensor_scalar_mul(
                    stat_c[tp][:, 1, s:s + 1], m2c, float(TV)
                )

            # per-half all-reduce stand-in + BN coefficients.
            # lg layout: cols 2*tp..2*tp+1 local (sum, ssq); 4+2*tp global.
            p2sm = tc.tile_pool(name="small2", bufs=1)
            smpool2 = p2sm.__enter__()
            p2s = tc.tile_pool(name="ys", bufs=8)
            yspool = p2s.__enter__()
            inv_sb = [ivb2[:, tp:tp + 1] for tp in range(TP)]
            b2_sb = [ivb2[:, 2 + tp:3 + tp] for tp in range(TP)]

            def emit_ar_coef(tp):
                loc = lg[:, 2 * tp:2 * tp + 2]
                glob = lg[:, 4 + 2 * tp:6 + 2 * tp]
                for j in range(2):
                    nc.vector.tensor_reduce(
                        loc[:, j:j + 1],
                        stat_c[tp][:, j, :],
                        axis=mybir.AxisListType.X,
                        op=ALU.add,
                    )
                if single_core:
                    nc.vector.tensor_copy(glob, loc)
                else:
                    cin = dpool.tile([128, 2], F32, name=f"cin{tp}")
                    cout = dpool.tile([128, 2], F32, name=f"cout{tp}")
                    nc.sync.dma_start(cin[:], loc)
                    nc.gpsimd.collective_compute(
                        "AllReduce",
                        ALU.add,
                        replica_groups=[list(range(N_CORES))],
                        ins=[cin[:].opt()],
                        outs=[cout[:].opt()],
                    )
                    nc.sync.dma_start(glob, cout[:])
                # inv = gamma/sqrt(var+eps); b2 = beta - mu*inv
                scr = smpool2.tile([128, 4], F32, tag=f"scr{tp}")
                mu = scr[:, 0:1]
                ex2 = scr[:, 1:2]
                nc.vector.tensor_scalar_mul(scr[:, 0:2], glob, 1.0 / N_GLOBAL)
                musq = scr[:, 2:3]
                nc.vector.tensor_tensor(musq, mu, mu, op=ALU.mult)
                var = scr[:, 1:2]
                nc.vector.tensor_tensor(var, ex2, musq, op=ALU.subtract)
                nc.vector.tensor_scalar_add(var, var, BN_EPS)
                sq = scr[:, 2:3]
                nc.scalar.activation(sq, var, AF.Sqrt)
                rs = scr[:, 3:4]
                nc.vector.reciprocal(rs, sq)
                iv = ivb2[:, tp:tp + 1]
                nc.vector.tensor_tensor(
                    iv, rs, gb_sb[:, tp:tp + 1], op=ALU.mult)
                mi = scr[:, 2:3]
                nc.vector.tensor_tensor(mi, mu, iv, op=ALU.mult)
                nc.vector.tensor_tensor(
                    ivb2[:, 2 + tp:3 + tp], gb_sb[:, 2 + tp:3 + tp], mi,
                    op=ALU.subtract)

            pend = []

            def emit_ph2_ts(tp):
                # out += ob*inv + b2: DVE 4x pass now, accum DMAs deferred
                # so later compute isn't scheduled behind the whole stream
                for blk in range(tp, NB, 2):
                    s = blk // 2
                    ob = xb_t[blk]
                    c0 = 128 * tp
                    for ci in range(2):
                        cols = slice(3200 * ci, 3200 * ci + 3200)
                        ys = yspool.tile([128, 3200], BF16, tag="ys")
                        nc.vector.tensor_scalar(
                            ys[:], ob[:, cols], inv_sb[tp], b2_sb[tp],
                            op0=ALU.mult, op1=ALU.add,
                        )
                        pend.append((s, c0, ci, ys))

            def flush_accums(n):
                # accum DMAs >1600 cols corrupt (hw-verified): two 1600-col
                # transfers per 3200-col chunk
                for _ in range(min(n, len(pend))):
                    s, c0, ci, ys = pend.pop(0)
                    for h in range(2):
                        t0 = 128 * ci + 64 * h
                        nc.gpsimd.dma_start(
                            out_d[s, c0:c0 + 128, t0:t0 + 64, :],
                            ys[:, 1600 * h:1600 * h + 1600].rearrange(
                                "p (a b) -> p a b", a=64, b=V),
                            accum_op=ALU.add,
                        )

            ORDER = [0, 2, 4, 6, 1, 3, 5, 7]
            emit_pre(0, make_m1p(0))
            for i, blk in enumerate(ORDER):
                nxt = ORDER[i + 1] if i + 1 < NB else None
                emit_main(blk, nxt)
                if blk == 4:
                    emit_xout(1)
                elif blk == 6:
                    emit_ar_coef(0)
                    emit_ph2_ts(0)
                    emit_xout(3)
                elif blk == 1:
                    flush_accums(4)
                elif blk == 3:
                    flush_accums(2)
                    emit_xout(5)
                    flush_accums(2)
                elif blk == 5:
                    flush_accums(2)
                    emit_xout(7)
                    flush_accums(2)
                elif blk == 7:
                    flush_accums(4)
            emit_ar_coef(1)
            emit_ph2_ts(1)
            flush_accums(len(pend))

            p2s.__exit__(None, None, None)
            p2sm.__exit__(None, None, None)
            for pc in (p1h, p1g, p1f, p1e, p1cc, p1c, p1b):
                pc.__exit__(None, None, None)

    nc.compile()
    return nc


def _host_prep(x, A, Wq, Wk, alpha, Wg, gamma, beta):
    bf = ml_dtypes.bfloat16
    A_sum = A.sum(axis=0)
    A_phys = A_sum / np.clip(A_sum.sum(axis=-1, keepdims=True), 1e-6, None)
    scl = 1.0 / (T * d_k ** 0.25)

    xw = np.zeros((TP, 128, 128), np.float32)
    wqk = np.zeros((TP, 128, 112), np.float32)
    for tp in range(TP):
        for gi in range(2):
            g = 2 * tp + gi
            r = slice(64 * gi, 64 * gi + 64)
            xw[tp][r, r] = Wg[g].T
            wqk[tp][r, 64 * gi:64 * gi + 16] = scl * Wq[g].T
            wqk[tp][r, 64 * gi + 32:64 * gi + 48] = scl * Wk[g].T

    talpha = np.repeat(np.tanh(alpha)[None, :], V, axis=0).astype(np.float32)
    sel = np.zeros((V, 4 * CH), np.float32)
    for d in range(4):
        sel[:, CH * d + V * d:CH * d + V * d + V] = np.eye(V)
    gb4 = np.concatenate(
        [gamma.reshape(TP, 128).T, beta.reshape(TP, 128).T], axis=1
    ).astype(np.float32)
    common = {
        "sel": sel.astype(bf),
        "xw": xw.astype(bf),
        "wqk": wqk.astype(bf),
        "aphys": A_phys.astype(np.float32),
        "talpha": talpha,
        "ident": np.eye(V, dtype=np.float32),
        "gb4": gb4,
    }
    return common


def kernel(x, A, Wq, Wk, alpha, Wg, gamma, beta, _trace=False, _trace_kwargs=None):
    x = np.asarray(x, np.float32)
    common = _host_prep(
        x,
        np.asarray(A, np.float32),
        np.asarray(Wq, np.float32),
        np.asarray(Wk, np.float32),
        np.asarray(alpha, np.float32),
        np.asarray(Wg, np.float32),
        np.asarray(gamma, np.float32),
        np.asarray(beta, np.float32),
    )
    if "nc" not in _CACHE:
        _CACHE["nc"] = _build()
    nc = _CACHE["nc"]

    in_maps = []
    for ci in range(N_CORES):
        m = dict(common)
        m["x"] = np.ascontiguousarray(x[BL * ci:BL * ci + BL])
        in_maps.append(m)

    kw = {}
    if _trace:
        kw = dict(trace=True, trace_kwargs=_trace_kwargs or {})
    res = bass_utils.run_bass_kernel_spmd(
        nc, in_maps, core_ids=list(range(N_CORES)), **kw
    )
    out = np.concatenate([r["out"] for r in res.results], axis=0)
    _CACHE["last_result"] = res
    return out


# revision 41
# speedup vs baseline: 1.5612x; 1.0015x over previous
"""AdaptiveCTRGCN distributed Trainium2 kernel (8 NeuronCores, batch-parallel).

Shapes (hardcoded): x (32,256,256,25) f32, A (3,25,25), Wq/Wk (4,16,64),
alpha (4,), Wg (4,64,64), gamma/beta (256,).
Per core: 4 samples. Two channel-halves ("tilepairs") of 128 channels
(= 2 groups of 64). BatchNorm statistics all-reduced across the 8 cores.

v4: gpsimd casting DMAs move x f32->bf16 straight into resident SBUF tiles.
Phase 1 also copies x -> out via DRAM->DRAM DMA during idle DMA time; the
conv output ob overwrites the input tile in place for every block. Phase 2
computes z = ob*inv+b2 in DVE 4x bf16 mode and DMA-accumulates z into the
output (out += z), so no spill, no re-read, and no residual-add pass.
The adjacency chain of block b+1 is software-pipelined into block b's
matmul loop.
"""
import sys

sys.path.insert(0, "/opt/trn_rl_repo")

import numpy as np
import ml_dtypes
from concourse import bass, bacc, tile, mybir, bass_utils

F32 = mybir.dt.float32
BF16 = mybir.dt.bfloat16
AF = mybir.ActivationFunctionType
ALU = mybir.AluOpType

N_CORES = 8
B, C, T, V = 32, 256, 256, 25
G, C_g, d_k = 4, 64, 16
BL = B // N_CORES          # samples per core = 4
TP = 2                     # channel halves (128 ch each)
NB = 2 * BL                # blocks per core = 8
QC = 16                    # 400-col quad-chunk groups per half (16 t each)
QW = 400                   # cols per quad-chunk (16 t * 25 v)
CH = 100                   # cols per matmul chunk (4 t * 25 v)
PW = 800                   # cols per m1 pair (2 quad-chunks)
TV = T * V                 # 6400
N_GLOBAL = float(B * T * V)   # BN sample count per channel
BN_EPS = 1e-5

# engine for the PSUM->SBUF ob copy per quad-chunk ('a'=ACT, 'd'=DVE)
OB_ENG = ['a', 'd', 'a', 'd', 'a', 'd', 'a', 'd',
          'a', 'd', 'a', 'd', 'a', 'a', 'a', 'a']

_CACHE = {}


def _build(single_core=False):
    nc = bacc.Bacc(
        "TRN2", target_bir_lowering=False, debug=False,
        num_devices=1 if single_core else N_CORES,
    )

    x_d = nc.dram_tensor("x", [BL, C, T, V], F32, kind="ExternalInput").ap()
    xw_d = nc.dram_tensor("xw", [TP, 128, 128], BF16, kind="ExternalInput").ap()
    wqk_d = nc.dram_tensor("wqk", [TP, 128, 112], BF16, kind="ExternalInput").ap()
    aphys_d = nc.dram_tensor("aphys", [V, V], F32, kind="ExternalInput").ap()
    talpha_d = nc.dram_tensor("talpha", [V, G], F32, kind="ExternalInput").ap()
    ident_d = nc.dram_tensor("ident", [V, V], F32, kind="ExternalInput").ap()
    sel_d = nc.dram_tensor("sel", [V, 4 * CH], BF16, kind="ExternalInput").ap()
    gb_d = nc.dram_tensor("gb4", [128, 4], F32, kind="ExternalInput").ap()
    out_d = nc.dram_tensor("out", [BL, C, T, V], F32, kind="ExternalOutput").ap()

    with tile.TileContext(nc) as tc:
        with (
            tc.tile_pool(name="const", bufs=1) as cpool,
            tc.tile_pool(name="dram", bufs=2, space="DRAM") as dpool,
        ):
            # ---- constants ----
            xw_sb = []
            wqk_sb = []
            for tp in range(TP):
                t1 = cpool.tile([128, 128], BF16, tag=f"xw{tp}")
                nc.sync.dma_start(t1[:], xw_d[tp])
                xw_sb.append(t1)
                t2 = cpool.tile([128, 112], BF16, tag=f"wqk{tp}")
                nc.sync.dma_start(t2[:], wqk_d[tp])
                wqk_sb.append(t2)
            gb_sb = cpool.tile([128, 4], F32, tag="gb4")
            nc.sync.dma_start(gb_sb[:], gb_d[:])
            aphys_sb = cpool.tile([V, V], F32, tag="aphys")
            nc.sync.dma_start(aphys_sb[:], aphys_d[:])
            talpha_sb = cpool.tile([V, G], F32, tag="talpha")
            nc.sync.dma_start(talpha_sb[:], talpha_d[:])
            ident_sb = cpool.tile([V, V], F32, tag="ident")
            nc.sync.dma_start(ident_sb[:], ident_d[:])
            sel_sb = cpool.tile([V, 4 * CH], BF16, tag="sel")
            nc.sync.dma_start(sel_sb[:], sel_d[:])

            # persistent per-block tiles (bf16): input copies, overwritten
            # in place by the conv output ob during the block's main loop.
            xb_t = [cpool.tile([128, TV], BF16, tag=f"xb{i}", name=f"xb{i}")
                    for i in range(NB)]
            stat_c = [cpool.tile([128, 2, BL], F32, tag=f"statc{tp}",
                                 name=f"statc{tp}")
                      for tp in range(TP)]
            lg = cpool.tile([128, 8], F32, tag="lg")
            ivb2 = cpool.tile([128, 4], F32, tag="ivb2")

            # ---- phase 1 pools ----
            p1b = tc.tile_pool(name="xwt", bufs=8)
            xwtpool = p1b.__enter__()
            p1c = tc.tile_pool(name="small", bufs=2)
            smpool = p1c.__enter__()
            p1cc = tc.tile_pool(name="bnc", bufs=2)
            bncpool = p1cc.__enter__()
            p1e = tc.tile_pool(name="mm1", bufs=2, space="PSUM")
            mm1pool = p1e.__enter__()
            p1f = tc.tile_pool(name="mm2", bufs=2, space="PSUM")
            mm2pool = p1f.__enter__()
            p1g = tc.tile_pool(name="qkp", bufs=1, space="PSUM")
            qkpool = p1g.__enter__()
            p1h = tc.tile_pool(name="smp", bufs=1, space="PSUM")
            smppool = p1h.__enter__()

            # all input loads up front: casting DMA f32->bf16 on gpsimd
            # (block 0 in 4 chunks so its qk pass starts early)
            for blk in range(NB):
                s, tp = blk // 2, blk % 2
                nch = 4 if blk == 0 else 1
                tpc = T // nch
                for hi in range(nch):
                    nc.gpsimd.dma_start(
                        xb_t[blk][:, TV // nch * hi:TV // nch * (hi + 1)]
                        .rearrange("p (a b) -> p a b", a=tpc, b=V),
                        x_d[s, 128 * tp:128 * tp + 128,
                            tpc * hi:tpc * (hi + 1), :],
                    )
            # residual: out = x, streamed DRAM->DRAM behind the input loads
            # on the same gpsimd queue (phase 2 accumulates ob*inv+b2 on top).
            # Emitted per channel-half so each half's accum stream can start
            # as soon as its own residual copies are done.
            def emit_xout(blk):
                s, tp = blk // 2, blk % 2
                nc.gpsimd.dma_start(
                    out_d[s, 128 * tp:128 * tp + 128, :, :],
                    x_d[s, 128 * tp:128 * tp + 128, :, :],
                )

            for blk in (0, 2, 4, 6):
                emit_xout(blk)

            # per-block pipelined state
            st = {}

            def emit_qk(blk, part):
                # 4 accumulating matmuls per call (part 0..3)
                if part == 0:
                    st.setdefault(blk, {})["qk"] = qkpool.tile(
                        [112, QW], F32, name="qkps")
                qk_ps = st[blk]["qk"]
                tp = blk % 2
                for qi in range(4 * part, 4 * part + 4):
                    nc.tensor.matmul(
                        qk_ps[:],
                        wqk_sb[tp][:],
                        xb_t[blk][:, QW * qi:QW * qi + QW],
                        start=(qi == 0),
                        stop=(qi == QC - 1),
                    )

            def emit_qkred(blk):
                qk_ps = st[blk]["qk"]
                for gi in range(2):
                    qt = smpool.tile([16, V], F32, tag=f"qg{gi}")
                    nc.vector.tensor_reduce(
                        qt[:],
                        qk_ps[64 * gi:64 * gi + 16, :].rearrange(
                            "p (t v) -> p v t", t=16, v=V
                        ),
                        axis=mybir.AxisListType.X,
                        op=ALU.add,
                    )
                    st[blk][f"qg{gi}"] = qt
                    kt = smpool.tile([16, V], F32, tag=f"kg{gi}")
                    nc.vector.tensor_reduce(
                        kt[:],
                        qk_ps[64 * gi + 32:64 * gi + 48, :].rearrange(
                            "p (t v) -> p v t", t=16, v=V
                        ),
                        axis=mybir.AxisListType.X,
                        op=ALU.add,
                    )
                    st[blk][f"kg{gi}"] = kt

            def emit_sc(blk, gi):
                # scores matmul + softmax chain -> ag (f32 [V,V])
                tp = blk % 2
                g = 2 * tp + gi
                sc_ps = smppool.tile([V, V], F32, tag="smt", name="scps")
                nc.tensor.matmul(
                    sc_ps[:],
                    st[blk][f"qg{gi}"][:],
                    st[blk][f"kg{gi}"][:],
                    start=True, stop=True,
                )
                mx = smpool.tile([V, 1], F32, tag=f"mx{gi}")
                nc.vector.tensor_reduce(
                    mx[:], sc_ps[:], axis=mybir.AxisListType.X,
                    op=ALU.max, negate=True,
                )
                nc.scalar.activation(sc_ps[:], sc_ps[:], AF.Exp, bias=mx[:])
                smrr = smpool.tile([V, 3], F32, tag=f"smrr{gi}")
                nc.vector.tensor_reduce(
                    smrr[:, 0:1], sc_ps[:], axis=mybir.AxisListType.X,
                    op=ALU.add
                )
                nc.vector.reciprocal(smrr[:, 1:2], smrr[:, 0:1])
                rst = smrr[:, 2:3]
                nc.vector.tensor_scalar_mul(
                    rst, smrr[:, 1:2], talpha_sb[:, g:g + 1]
                )
                ag = smpool.tile([V, V], F32, tag=f"ag{gi}")
                nc.vector.scalar_tensor_tensor(
                    ag[:], sc_ps[:], rst, aphys_sb[:],
                    op0=ALU.mult, op1=ALU.add,
                )
                st[blk][f"ag{gi}"] = ag

            def emit_tr(blk, gi):
                agt_ps = smppool.tile([V, V], F32, tag="smt", name="agtps")
                nc.tensor.transpose(agt_ps[:], st[blk][f"ag{gi}"][:], ident_sb[:])
                agtb = smpool.tile([V, V], BF16, tag=f"agtb{gi}")
                nc.vector.tensor_copy(agtb[:], agt_ps[:])
                st[blk][f"agtb{gi}"] = agtb

            def emit_i4(blk, gi):
                i4a_ps = smppool.tile([CH, CH], F32, tag="smt", name="i4aps")
                agtb = st[blk][f"agtb{gi}"]
                for d in range(4):
                    nc.tensor.matmul(
                        i4a_ps[:, V * d:V * d + V],
                        sel_sb[:, CH * d:CH * d + CH],
                        agtb[:],
                        start=True, stop=True,
                    )
                i4a = smpool.tile([CH, CH], BF16, tag=f"i4a{gi}")
                nc.scalar.activation(i4a[:], i4a_ps[:], AF.Copy)
                st[blk][f"i4a{gi}"] = i4a

            def emit_pre(blk, m1p=None):
                # standalone prologue for block 0: interleave its own m1
                # pairs so the PE has work while the softmax chain runs
                for part in range(4):
                    emit_qk(blk, part)
                emit_qkred(blk)
                steps = [
                    ("sc", 0), ("m1", 0), ("sc", 1), ("m1", 1),
                    ("tr", 0), ("m1", 2), ("tr", 1), ("m1", 3),
                    ("i4", 0), ("m1", 4), ("i4", 1), ("m1", 5),
                ]
                for kind, arg in steps:
                    if kind == "sc":
                        emit_sc(blk, arg)
                    elif kind == "tr":
                        emit_tr(blk, arg)
                    elif kind == "i4":
                        emit_i4(blk, arg)
                    elif m1p is not None:
                        m1p(arg)

            def make_m1p(blk):
                tp = blk % 2
                xb = xb_t[blk]
                xwt_p = st.setdefault(blk, {}).setdefault("xwt", {})

                def do_m1p(pi):
                    if pi in xwt_p:
                        return
                    mp = mm1pool.tile([CH, 1024], F32, name="mp")
                    for j in range(8):
                        nc.tensor.matmul(
                            mp[:, 128 * j:128 * j + 128],
                            xb[:, PW * pi + CH * j:PW * pi + CH * j + CH],
                            xw_sb[tp][:],
                            start=True, stop=True,
                        )
                    xwt = xwtpool.tile([CH, 1024], BF16, tag="xwt", name="xwt")
                    nc.scalar.activation(xwt[:], mp[:], AF.Copy)
                    xwt_p[pi] = xwt

                return do_m1p

            def emit_main(blk, nxt):
                """m1/m2/copies/bn for blk; interleaves PRE of nxt."""
                tp = blk % 2
                xb = xb_t[blk]
                ob = xb_t[blk]
                do_m1p = make_m1p(blk)
                xwt_p = st[blk]["xwt"]

                do_m1p(0)
                do_m1p(1)
                bnc = bncpool.tile([128, QC, 6], F32, tag="bnc")
                i4a = [st[blk]["i4a0"], st[blk]["i4a1"]]
                for qi in range(QC):
                    pi = qi // 2
                    if qi % 2 == 0 and pi + 2 <= 7:
                        do_m1p(pi + 2)
                    xwt = xwt_p[pi]
                    base = 512 * (qi % 2)
                    op = mm2pool.tile([128, QW], F32)
                    for h in range(4):
                        for gi in range(2):
                            nc.tensor.matmul(
                                op[64 * gi:64 * gi + 64, CH * h:CH * h + CH],
                                xwt[:, base + 128 * h + 64 * gi:
                                    base + 128 * h + 64 * gi + 64],
                                i4a[gi][:],
                                start=True, stop=True,
                            )
                    dst = ob[:, QW * qi:QW * qi + QW]
                    if OB_ENG[qi] == 'd':
                        nc.vector.tensor_copy(dst, op[:])
                    else:
                        nc.scalar.activation(dst, op[:], AF.Copy)
                    lag = 0 if blk == NB - 1 else 3
                    if qi >= lag:
                        qj = qi - lag
                        nc.vector.bn_stats(
                            bnc[:, qj, :], ob[:, QW * qj:QW * qj + QW]
                        )
                    # interleave next block's qk/adjacency chain
                    if nxt is not None:
                        if qi < 4:
                            emit_qk(nxt, qi)
                        elif qi == 4:
                            emit_qkred(nxt)
                        elif qi == 5:
                            emit_sc(nxt, 0)
                        elif qi == 6:
                            emit_sc(nxt, 1)
                        elif qi == 9:
                            emit_tr(nxt, 0)
                        elif qi == 10:
                            emit_tr(nxt, 1)
                        elif qi == 12:
                            emit_i4(nxt, 0)
                        elif qi == 13:
                            emit_i4(nxt, 1)
                if blk != NB - 1:
                    for qj in (QC - 2, QC - 1):
                        nc.vector.bn_stats(
                            bnc[:, qj, :], ob[:, QW * qj:QW * qj + QW]
                        )

                # block stats -> sum/ssq columns
                s = blk // 2
                msv = smpool.tile([128, 4], F32, tag="msv")
                nc.vector.bn_aggr(
                    msv[:, 0:2], bnc[:].rearrange("p a b -> p (a b)")
                )
                nc.vector.tensor_scalar_mul(
                    stat_c[tp][:, 0, s:s + 1], msv[:, 0:1], float(TV)
                )
                m2c = msv[:, 2:3]
                nc.vector.tensor_tensor(
                    m2c, msv[:, 0:1], msv[:, 0:1], op=ALU.mult
                )
                nc.vector.tensor_tensor(m2c, m2c, msv[:, 1:2], op=ALU.add)
                nc.vector.tensor_scalar_mul(
                    stat_c[tp][:, 1, s:s + 1], m2c, float(TV)
                )

            # per-half all-reduce stand-in + BN coefficients.
            # lg layout: cols 2*tp..2*tp+1 local (sum, ssq); 4+2*tp global.
            p2sm = tc.tile_pool(name="small2", bufs=1)
            smpool2 = p2sm.__enter__()
            p2s = tc.tile_pool(name="ys", bufs=8)
            yspool = p2s.__enter__()
            inv_sb = [ivb2[:, tp:tp + 1] for tp in range(TP)]
            b2_sb = [ivb2[:, 2 + tp:3 + tp] for tp in range(TP)]

            def emit_ar_coef(tp):
                loc = lg[:, 2 * tp:2 * tp + 2]
                glob = lg[:, 4 + 2 * tp:6 + 2 * tp]
                red_dst = glob if single_core else loc
                for j in range(2):
                    nc.vector.tensor_reduce(
                        red_dst[:, j:j + 1],
                        stat_c[tp][:, j, :],
                        axis=mybir.AxisListType.X,
                        op=ALU.add,
                    )
                if single_core:
                    pass
                else:
                    cin = dpool.tile([128, 2], F32, name=f"cin{tp}")
                    cout = dpool.tile([128, 2], F32, name=f"cout{tp}")
                    nc.sync.dma_start(cin[:], loc)
                    nc.gpsimd.collective_compute(
                        "AllReduce",
                        ALU.add,
                        replica_groups=[list(range(N_CORES))],
                        ins=[cin[:].opt()],
                        outs=[cout[:].opt()],
                    )
                    nc.sync.dma_start(glob, cout[:])
                # inv = gamma/sqrt(var+eps); b2 = beta - mu*inv
                scr = smpool2.tile([128, 4], F32, tag=f"scr{tp}")
                mu = scr[:, 0:1]
                ex2 = scr[:, 1:2]
                nc.vector.tensor_scalar_mul(scr[:, 0:2], glob, 1.0 / N_GLOBAL)
                musq = scr[:, 2:3]
                nc.vector.tensor_tensor(musq, mu, mu, op=ALU.mult)
                var = scr[:, 1:2]
                nc.vector.tensor_tensor(var, ex2, musq, op=ALU.subtract)
                nc.vector.tensor_scalar_add(var, var, BN_EPS)
                sq = scr[:, 2:3]
                nc.scalar.activation(sq, var, AF.Sqrt)
                rs = scr[:, 3:4]
                nc.vector.reciprocal(rs, sq)
                iv = ivb2[:, tp:tp + 1]
                nc.vector.tensor_tensor(
                    iv, rs, gb_sb[:, tp:tp + 1], op=ALU.mult)
                mi = scr[:, 2:3]
                nc.vector.tensor_tensor(mi, mu, iv, op=ALU.mult)
                nc.vector.tensor_tensor(
                    ivb2[:, 2 + tp:3 + tp], gb_sb[:, 2 + tp:3 + tp], mi,
                    op=ALU.subtract)

            pend = []

            def emit_ph2_ts(tp):
                # out += ob*inv + b2: DVE 4x pass now, accum DMAs deferred
                # so later compute isn't scheduled behind the whole stream
                first = True
                for blk in range(tp, NB, 2):
                    s = blk // 2
                    ob = xb_t[blk]
                    c0 = 128 * tp
                    for ci in range(2):
                        cols = slice(3200 * ci, 3200 * ci + 3200)
                        ys = yspool.tile([128, 3200], BF16, tag="ys")
                        if first:
                            # split the first chunk so its first 1600-col
                            # accum can fire before the whole ts finishes
                            for h2 in range(2):
                                nc.vector.tensor_scalar(
                                    ys[:, 1600 * h2:1600 * h2 + 1600],
                                    ob[:, 3200 * ci + 1600 * h2:
                                       3200 * ci + 1600 * h2 + 1600],
                                    inv_sb[tp], b2_sb[tp],
                                    op0=ALU.mult, op1=ALU.add,
                                )
                            first = False
                        else:
                            nc.vector.tensor_scalar(
                                ys[:], ob[:, cols], inv_sb[tp], b2_sb[tp],
                                op0=ALU.mult, op1=ALU.add,
                            )
                        pend.append((s, c0, ci, ys))

            def flush_accums(n):
                # accum DMAs >1600 cols corrupt (hw-verified): two 1600-col
                # transfers per 3200-col chunk
                for _ in range(min(n, len(pend))):
                    s, c0, ci, ys = pend.pop(0)
                    for h in range(2):
                        t0 = 128 * ci + 64 * h
                        nc.gpsimd.dma_start(
                            out_d[s, c0:c0 + 128, t0:t0 + 64, :],
                            ys[:, 1600 * h:1600 * h + 1600].rearrange(
                                "p (a b) -> p a b", a=64, b=V),
                            accum_op=ALU.add,
                        )

            ORDER = [0, 2, 4, 6, 1, 3, 5, 7]
            emit_pre(0, make_m1p(0))
            for i, blk in enumerate(ORDER):
                nxt = ORDER[i + 1] if i + 1 < NB else None
                emit_main(blk, nxt)
                if blk == 4:
                    emit_xout(1)
                elif blk == 6:
                    emit_ar_coef(0)
                    emit_ph2_ts(0)
                    emit_xout(3)
                elif blk == 1:
                    flush_accums(4)
                elif blk == 3:
                    flush_accums(2)
                    emit_xout(5)
                    flush_accums(2)
                elif blk == 5:
                    flush_accums(2)
                    emit_xout(7)
                    flush_accums(2)
                elif blk == 7:
                    flush_accums(4)
            emit_ar_coef(1)
            emit_ph2_ts(1)
            flush_accums(len(pend))

            p2s.__exit__(None, None, None)
            p2sm.__exit__(None, None, None)
            for pc in (p1h, p1g, p1f, p1e, p1cc, p1c, p1b):
                pc.__exit__(None, None, None)

    nc.compile()
    return nc


def _host_prep(x, A, Wq, Wk, alpha, Wg, gamma, beta):
    bf = ml_dtypes.bfloat16
    A_sum = A.sum(axis=0)
    A_phys = A_sum / np.clip(A_sum.sum(axis=-1, keepdims=True), 1e-6, None)
    scl = 1.0 / (T * d_k ** 0.25)

    xw = np.zeros((TP, 128, 128), np.float32)
    wqk = np.zeros((TP, 128, 112), np.float32)
    for tp in range(TP):
        for gi in range(2):
            g = 2 * tp + gi
            r = slice(64 * gi, 64 * gi + 64)
            xw[tp][r, r] = Wg[g].T
            wqk[tp][r, 64 * gi:64 * gi + 16] = scl * Wq[g].T
            wqk[tp][r, 64 * gi + 32:64 * gi + 48] = scl * Wk[g].T

    talpha = np.repeat(np.tanh(alpha)[None, :], V, axis=0).astype(np.float32)
    sel = np.zeros((V, 4 * CH), np.float32)
    for d in range(4):
        sel[:, CH * d + V * d:CH * d + V * d + V] = np.eye(V)
    gb4 = np.concatenate(
        [gamma.reshape(TP, 128).T, beta.reshape(TP, 128).T], axis=1
    ).astype(np.float32)
    common = {
        "sel": sel.astype(bf),
        "xw": xw.astype(bf),
        "wqk": wqk.astype(bf),
        "aphys": A_phys.astype(np.float32),
        "talpha": talpha,
        "ident": np.eye(V, dtype=np.float32),
        "gb4": gb4,
    }
    return common


def kernel(x, A, Wq, Wk, alpha, Wg, gamma, beta, _trace=False, _trace_kwargs=None):
    x = np.asarray(x, np.float32)
    common = _host_prep(
        x,
        np.asarray(A, np.float32),
        np.asarray(Wq, np.float32),
        np.asarray(Wk, np.float32),
        np.asarray(alpha, np.float32),
        np.asarray(Wg, np.float32),
        np.asarray(gamma, np.float32),
        np.asarray(beta, np.float32),
    )
    if "nc" not in _CACHE:
        _CACHE["nc"] = _build()
    nc = _CACHE["nc"]

    in_maps = []
    for ci in range(N_CORES):
        m = dict(common)
        m["x"] = np.ascontiguousarray(x[BL * ci:BL * ci + BL])
        in_maps.append(m)

    kw = {}
    if _trace:
        kw = dict(trace=True, trace_kwargs=_trace_kwargs or {})
    res = bass_utils.run_bass_kernel_spmd(
        nc, in_maps, core_ids=list(range(N_CORES)), **kw
    )
    out = np.concatenate([r["out"] for r in res.results], axis=0)
    _CACHE["last_result"] = res
    return out


# revision 42
# speedup vs baseline: 1.5686x; 1.0047x over previous
"""AdaptiveCTRGCN distributed Trainium2 kernel (8 NeuronCores, batch-parallel).

Shapes (hardcoded): x (32,256,256,25) f32, A (3,25,25), Wq/Wk (4,16,64),
alpha (4,), Wg (4,64,64), gamma/beta (256,).
Per core: 4 samples. Two channel-halves ("tilepairs") of 128 channels
(= 2 groups of 64). BatchNorm statistics all-reduced across the 8 cores.

v4: gpsimd casting DMAs move x f32->bf16 straight into resident SBUF tiles.
Phase 1 also copies x -> out via DRAM->DRAM DMA during idle DMA time; the
conv output ob overwrites the input tile in place for every block. Phase 2
computes z = ob*inv+b2 in DVE 4x bf16 mode and DMA-accumulates z into the
output (out += z), so no spill, no re-read, and no residual-add pass.
The adjacency chain of block b+1 is software-pipelined into block b's
matmul loop.
"""
import sys

sys.path.insert(0, "/opt/trn_rl_repo")

import numpy as np
import ml_dtypes
from concourse import bass, bacc, tile, mybir, bass_utils

F32 = mybir.dt.float32
BF16 = mybir.dt.bfloat16
AF = mybir.ActivationFunctionType
ALU = mybir.AluOpType

N_CORES = 8
B, C, T, V = 32, 256, 256, 25
G, C_g, d_k = 4, 64, 16
BL = B // N_CORES          # samples per core = 4
TP = 2                     # channel halves (128 ch each)
NB = 2 * BL                # blocks per core = 8
QC = 16                    # 400-col quad-chunk groups per half (16 t each)
QW = 400                   # cols per quad-chunk (16 t * 25 v)
CH = 100                   # cols per matmul chunk (4 t * 25 v)
PW = 800                   # cols per m1 pair (2 quad-chunks)
TV = T * V                 # 6400
N_GLOBAL = float(B * T * V)   # BN sample count per channel
BN_EPS = 1e-5

# engine for the PSUM->SBUF ob copy per quad-chunk ('a'=ACT, 'd'=DVE)
OB_ENG = ['a', 'd', 'a', 'd', 'a', 'd', 'a', 'd',
          'a', 'd', 'a', 'd', 'a', 'a', 'a', 'a']

_CACHE = {}


def _build(single_core=False):
    nc = bacc.Bacc(
        "TRN2", target_bir_lowering=False, debug=False,
        num_devices=1 if single_core else N_CORES,
    )

    x_d = nc.dram_tensor("x", [BL, C, T, V], F32, kind="ExternalInput").ap()
    xw_d = nc.dram_tensor("xw", [TP, 128, 128], BF16, kind="ExternalInput").ap()
    wqk_d = nc.dram_tensor("wqk", [TP, 128, 112], BF16, kind="ExternalInput").ap()
    aphys_d = nc.dram_tensor("aphys", [V, V], F32, kind="ExternalInput").ap()
    talpha_d = nc.dram_tensor("talpha", [V, G], F32, kind="ExternalInput").ap()
    ident_d = nc.dram_tensor("ident", [V, V], F32, kind="ExternalInput").ap()
    sel_d = nc.dram_tensor("sel", [V, 4 * CH], BF16, kind="ExternalInput").ap()
    gb_d = nc.dram_tensor("gb4", [128, 4], F32, kind="ExternalInput").ap()
    out_d = nc.dram_tensor("out", [BL, C, T, V], F32, kind="ExternalOutput").ap()

    with tile.TileContext(nc) as tc:
        with (
            tc.tile_pool(name="const", bufs=1) as cpool,
            tc.tile_pool(name="dram", bufs=2, space="DRAM") as dpool,
        ):
            # ---- constants ----
            xw_sb = []
            wqk_sb = []
            for tp in range(TP):
                t1 = cpool.tile([128, 128], BF16, tag=f"xw{tp}")
                nc.sync.dma_start(t1[:], xw_d[tp])
                xw_sb.append(t1)
                t2 = cpool.tile([128, 112], BF16, tag=f"wqk{tp}")
                nc.sync.dma_start(t2[:], wqk_d[tp])
                wqk_sb.append(t2)
            gb_sb = cpool.tile([128, 4], F32, tag="gb4")
            nc.sync.dma_start(gb_sb[:], gb_d[:])
            aphys_sb = cpool.tile([V, V], F32, tag="aphys")
            nc.sync.dma_start(aphys_sb[:], aphys_d[:])
            talpha_sb = cpool.tile([V, G], F32, tag="talpha")
            nc.sync.dma_start(talpha_sb[:], talpha_d[:])
            ident_sb = cpool.tile([V, V], F32, tag="ident")
            nc.sync.dma_start(ident_sb[:], ident_d[:])
            sel_sb = cpool.tile([V, 4 * CH], BF16, tag="sel")
            nc.sync.dma_start(sel_sb[:], sel_d[:])

            # persistent per-block tiles (bf16): input copies, overwritten
            # in place by the conv output ob during the block's main loop.
            xb_t = [cpool.tile([128, TV], BF16, tag=f"xb{i}", name=f"xb{i}")
                    for i in range(NB)]
            stat_c = [cpool.tile([128, 2, BL], F32, tag=f"statc{tp}",
                                 name=f"statc{tp}")
                      for tp in range(TP)]
            lg = cpool.tile([128, 8], F32, tag="lg")
            ivb2 = cpool.tile([128, 4], F32, tag="ivb2")

            # ---- phase 1 pools ----
            p1b = tc.tile_pool(name="xwt", bufs=8)
            xwtpool = p1b.__enter__()
            p1c = tc.tile_pool(name="small", bufs=2)
            smpool = p1c.__enter__()
            p1cc = tc.tile_pool(name="bnc", bufs=2)
            bncpool = p1cc.__enter__()
            p1e = tc.tile_pool(name="mm1", bufs=2, space="PSUM")
            mm1pool = p1e.__enter__()
            p1f = tc.tile_pool(name="mm2", bufs=2, space="PSUM")
            mm2pool = p1f.__enter__()
            p1g = tc.tile_pool(name="qkp", bufs=1, space="PSUM")
            qkpool = p1g.__enter__()
            p1h = tc.tile_pool(name="smp", bufs=1, space="PSUM")
            smppool = p1h.__enter__()

            # all input loads up front: casting DMA f32->bf16 on gpsimd
            # (block 0 in 4 chunks so its qk pass starts early)
            for blk in range(NB):
                s, tp = blk // 2, blk % 2
                nch = 4 if blk in (0, 2) else 1
                tpc = T // nch
                for hi in range(nch):
                    nc.gpsimd.dma_start(
                        xb_t[blk][:, TV // nch * hi:TV // nch * (hi + 1)]
                        .rearrange("p (a b) -> p a b", a=tpc, b=V),
                        x_d[s, 128 * tp:128 * tp + 128,
                            tpc * hi:tpc * (hi + 1), :],
                    )
            # residual: out = x, streamed DRAM->DRAM behind the input loads
            # on the same gpsimd queue (phase 2 accumulates ob*inv+b2 on top).
            # Emitted per channel-half so each half's accum stream can start
            # as soon as its own residual copies are done.
            def emit_xout(blk):
                s, tp = blk // 2, blk % 2
                nc.gpsimd.dma_start(
                    out_d[s, 128 * tp:128 * tp + 128, :, :],
                    x_d[s, 128 * tp:128 * tp + 128, :, :],
                )

            for blk in (0, 2, 4, 6):
                emit_xout(blk)

            # per-block pipelined state
            st = {}

            def emit_qk(blk, part):
                # 4 accumulating matmuls per call (part 0..3)
                if part == 0:
                    st.setdefault(blk, {})["qk"] = qkpool.tile(
                        [112, QW], F32, name="qkps")
                qk_ps = st[blk]["qk"]
                tp = blk % 2
                for qi in range(4 * part, 4 * part + 4):
                    nc.tensor.matmul(
                        qk_ps[:],
                        wqk_sb[tp][:],
                        xb_t[blk][:, QW * qi:QW * qi + QW],
                        start=(qi == 0),
                        stop=(qi == QC - 1),
                    )

            def emit_qkred(blk):
                qk_ps = st[blk]["qk"]
                for gi in range(2):
                    qt = smpool.tile([16, V], F32, tag=f"qg{gi}")
                    nc.vector.tensor_reduce(
                        qt[:],
                        qk_ps[64 * gi:64 * gi + 16, :].rearrange(
                            "p (t v) -> p v t", t=16, v=V
                        ),
                        axis=mybir.AxisListType.X,
                        op=ALU.add,
                    )
                    st[blk][f"qg{gi}"] = qt
                    kt = smpool.tile([16, V], F32, tag=f"kg{gi}")
                    nc.vector.tensor_reduce(
                        kt[:],
                        qk_ps[64 * gi + 32:64 * gi + 48, :].rearrange(
                            "p (t v) -> p v t", t=16, v=V
                        ),
                        axis=mybir.AxisListType.X,
                        op=ALU.add,
                    )
                    st[blk][f"kg{gi}"] = kt

            def emit_sc(blk, gi):
                # scores matmul + softmax chain -> ag (f32 [V,V])
                tp = blk % 2
                g = 2 * tp + gi
                sc_ps = smppool.tile([V, V], F32, tag="smt", name="scps")
                nc.tensor.matmul(
                    sc_ps[:],
                    st[blk][f"qg{gi}"][:],
                    st[blk][f"kg{gi}"][:],
                    start=True, stop=True,
                )
                mx = smpool.tile([V, 1], F32, tag=f"mx{gi}")
                nc.vector.tensor_reduce(
                    mx[:], sc_ps[:], axis=mybir.AxisListType.X,
                    op=ALU.max, negate=True,
                )
                nc.scalar.activation(sc_ps[:], sc_ps[:], AF.Exp, bias=mx[:])
                smrr = smpool.tile([V, 3], F32, tag=f"smrr{gi}")
                nc.vector.tensor_reduce(
                    smrr[:, 0:1], sc_ps[:], axis=mybir.AxisListType.X,
                    op=ALU.add
                )
                nc.vector.reciprocal(smrr[:, 1:2], smrr[:, 0:1])
                rst = smrr[:, 2:3]
                nc.vector.tensor_scalar_mul(
                    rst, smrr[:, 1:2], talpha_sb[:, g:g + 1]
                )
                ag = smpool.tile([V, V], F32, tag=f"ag{gi}")
                nc.vector.scalar_tensor_tensor(
                    ag[:], sc_ps[:], rst, aphys_sb[:],
                    op0=ALU.mult, op1=ALU.add,
                )
                st[blk][f"ag{gi}"] = ag

            def emit_tr(blk, gi):
                agt_ps = smppool.tile([V, V], F32, tag="smt", name="agtps")
                nc.tensor.transpose(agt_ps[:], st[blk][f"ag{gi}"][:], ident_sb[:])
                agtb = smpool.tile([V, V], BF16, tag=f"agtb{gi}")
                nc.vector.tensor_copy(agtb[:], agt_ps[:])
                st[blk][f"agtb{gi}"] = agtb

            def emit_i4(blk, gi):
                i4a_ps = smppool.tile([CH, CH], F32, tag="smt", name="i4aps")
                agtb = st[blk][f"agtb{gi}"]
                for d in range(4):
                    nc.tensor.matmul(
                        i4a_ps[:, V * d:V * d + V],
                        sel_sb[:, CH * d:CH * d + CH],
                        agtb[:],
                        start=True, stop=True,
                    )
                i4a = smpool.tile([CH, CH], BF16, tag=f"i4a{gi}")
                nc.scalar.activation(i4a[:], i4a_ps[:], AF.Copy)
                st[blk][f"i4a{gi}"] = i4a

            def emit_pre(blk, m1p=None):
                # standalone prologue for block 0: interleave its own m1
                # pairs so the PE has work while the softmax chain runs
                for part in range(4):
                    emit_qk(blk, part)
                emit_qkred(blk)
                steps = [
                    ("sc", 0), ("m1", 0), ("sc", 1), ("m1", 1),
                    ("tr", 0), ("m1", 2), ("tr", 1), ("m1", 3),
                    ("i4", 0), ("m1", 4), ("i4", 1), ("m1", 5),
                ]
                for kind, arg in steps:
                    if kind == "sc":
                        emit_sc(blk, arg)
                    elif kind == "tr":
                        emit_tr(blk, arg)
                    elif kind == "i4":
                        emit_i4(blk, arg)
                    elif m1p is not None:
                        m1p(arg)

            def make_m1p(blk):
                tp = blk % 2
                xb = xb_t[blk]
                xwt_p = st.setdefault(blk, {}).setdefault("xwt", {})

                def do_m1p(pi):
                    if pi in xwt_p:
                        return
                    mp = mm1pool.tile([CH, 1024], F32, name="mp")
                    for j in range(8):
                        nc.tensor.matmul(
                            mp[:, 128 * j:128 * j + 128],
                            xb[:, PW * pi + CH * j:PW * pi + CH * j + CH],
                            xw_sb[tp][:],
                            start=True, stop=True,
                        )
                    xwt = xwtpool.tile([CH, 1024], BF16, tag="xwt", name="xwt")
                    nc.scalar.activation(xwt[:], mp[:], AF.Copy)
                    xwt_p[pi] = xwt

                return do_m1p

            def emit_main(blk, nxt):
                """m1/m2/copies/bn for blk; interleaves PRE of nxt."""
                tp = blk % 2
                xb = xb_t[blk]
                ob = xb_t[blk]
                do_m1p = make_m1p(blk)
                xwt_p = st[blk]["xwt"]

                do_m1p(0)
                do_m1p(1)
                bnc = bncpool.tile([128, QC, 6], F32, tag="bnc")
                i4a = [st[blk]["i4a0"], st[blk]["i4a1"]]
                for qi in range(QC):
                    pi = qi // 2
                    if qi % 2 == 0 and pi + 2 <= 7:
                        do_m1p(pi + 2)
                    xwt = xwt_p[pi]
                    base = 512 * (qi % 2)
                    op = mm2pool.tile([128, QW], F32)
                    for h in range(4):
                        for gi in range(2):
                            nc.tensor.matmul(
                                op[64 * gi:64 * gi + 64, CH * h:CH * h + CH],
                                xwt[:, base + 128 * h + 64 * gi:
                                    base + 128 * h + 64 * gi + 64],
                                i4a[gi][:],
                                start=True, stop=True,
                            )
                    dst = ob[:, QW * qi:QW * qi + QW]
                    if OB_ENG[qi] == 'd':
                        nc.vector.tensor_copy(dst, op[:])
                    else:
                        nc.scalar.activation(dst, op[:], AF.Copy)
                    lag = 0 if blk == NB - 1 else 3
                    if qi >= lag:
                        qj = qi - lag
                        nc.vector.bn_stats(
                            bnc[:, qj, :], ob[:, QW * qj:QW * qj + QW]
                        )
                    # interleave next block's qk/adjacency chain
                    if nxt is not None:
                        if qi < 4:
                            emit_qk(nxt, qi)
                        elif qi == 4:
                            emit_qkred(nxt)
                        elif qi == 5:
                            emit_sc(nxt, 0)
                        elif qi == 6:
                            emit_sc(nxt, 1)
                        elif qi == 9:
                            emit_tr(nxt, 0)
                        elif qi == 10:
                            emit_tr(nxt, 1)
                        elif qi == 12:
                            emit_i4(nxt, 0)
                        elif qi == 13:
                            emit_i4(nxt, 1)
                if blk != NB - 1:
                    for qj in (QC - 2, QC - 1):
                        nc.vector.bn_stats(
                            bnc[:, qj, :], ob[:, QW * qj:QW * qj + QW]
                        )

                # block stats -> sum/ssq columns
                s = blk // 2
                msv = smpool.tile([128, 4], F32, tag="msv")
                nc.vector.bn_aggr(
                    msv[:, 0:2], bnc[:].rearrange("p a b -> p (a b)")
                )
                nc.vector.tensor_scalar_mul(
                    stat_c[tp][:, 0, s:s + 1], msv[:, 0:1], float(TV)
                )
                m2c = msv[:, 2:3]
                nc.vector.tensor_tensor(
                    m2c, msv[:, 0:1], msv[:, 0:1], op=ALU.mult
                )
                nc.vector.tensor_tensor(m2c, m2c, msv[:, 1:2], op=ALU.add)
                nc.vector.tensor_scalar_mul(
                    stat_c[tp][:, 1, s:s + 1], m2c, float(TV)
                )

            # per-half all-reduce stand-in + BN coefficients.
            # lg layout: cols 2*tp..2*tp+1 local (sum, ssq); 4+2*tp global.
            p2sm = tc.tile_pool(name="small2", bufs=1)
            smpool2 = p2sm.__enter__()
            p2s = tc.tile_pool(name="ys", bufs=8)
            yspool = p2s.__enter__()
            inv_sb = [ivb2[:, tp:tp + 1] for tp in range(TP)]
            b2_sb = [ivb2[:, 2 + tp:3 + tp] for tp in range(TP)]

            def emit_ar_coef(tp):
                loc = lg[:, 2 * tp:2 * tp + 2]
                glob = lg[:, 4 + 2 * tp:6 + 2 * tp]
                red_dst = glob if single_core else loc
                for j in range(2):
                    nc.vector.tensor_reduce(
                        red_dst[:, j:j + 1],
                        stat_c[tp][:, j, :],
                        axis=mybir.AxisListType.X,
                        op=ALU.add,
                    )
                if single_core:
                    pass
                else:
                    cin = dpool.tile([128, 2], F32, name=f"cin{tp}")
                    cout = dpool.tile([128, 2], F32, name=f"cout{tp}")
                    nc.sync.dma_start(cin[:], loc)
                    nc.gpsimd.collective_compute(
                        "AllReduce",
                        ALU.add,
                        replica_groups=[list(range(N_CORES))],
                        ins=[cin[:].opt()],
                        outs=[cout[:].opt()],
                    )
                    nc.sync.dma_start(glob, cout[:])
                # inv = gamma/sqrt(var+eps); b2 = beta - mu*inv
                scr = smpool2.tile([128, 4], F32, tag=f"scr{tp}")
                mu = scr[:, 0:1]
                ex2 = scr[:, 1:2]
                nc.vector.tensor_scalar_mul(scr[:, 0:2], glob, 1.0 / N_GLOBAL)
                musq = scr[:, 2:3]
                nc.vector.tensor_tensor(musq, mu, mu, op=ALU.mult)
                var = scr[:, 1:2]
                nc.vector.tensor_tensor(var, ex2, musq, op=ALU.subtract)
                nc.vector.tensor_scalar_add(var, var, BN_EPS)
                sq = scr[:, 2:3]
                nc.scalar.activation(sq, var, AF.Sqrt)
                rs = scr[:, 3:4]
                nc.vector.reciprocal(rs, sq)
                iv = ivb2[:, tp:tp + 1]
                nc.vector.tensor_tensor(
                    iv, rs, gb_sb[:, tp:tp + 1], op=ALU.mult)
                mi = scr[:, 2:3]
                nc.vector.tensor_tensor(mi, mu, iv, op=ALU.mult)
                nc.vector.tensor_tensor(
                    ivb2[:, 2 + tp:3 + tp], gb_sb[:, 2 + tp:3 + tp], mi,
                    op=ALU.subtract)

            pend = []

            def emit_ph2_ts(tp):
                # out += ob*inv + b2: DVE 4x pass now, accum DMAs deferred
                # so later compute isn't scheduled behind the whole stream
                first = True
                for blk in range(tp, NB, 2):
                    s = blk // 2
                    ob = xb_t[blk]
                    c0 = 128 * tp
                    for ci in range(2):
                        cols = slice(3200 * ci, 3200 * ci + 3200)
                        ys = yspool.tile([128, 3200], BF16, tag="ys")
                        if first:
                            # split the first chunk so its first 1600-col
                            # accum can fire before the whole ts finishes
                            for h2 in range(2):
                                nc.vector.tensor_scalar(
                                    ys[:, 1600 * h2:1600 * h2 + 1600],
                                    ob[:, 3200 * ci + 1600 * h2:
                                       3200 * ci + 1600 * h2 + 1600],
                                    inv_sb[tp], b2_sb[tp],
                                    op0=ALU.mult, op1=ALU.add,
                                )
                            first = False
                        else:
                            nc.vector.tensor_scalar(
                                ys[:], ob[:, cols], inv_sb[tp], b2_sb[tp],
                                op0=ALU.mult, op1=ALU.add,
                            )
                        pend.append((s, c0, ci, ys))

            def flush_accums(n):
                # accum DMAs >1600 cols corrupt (hw-verified): two 1600-col
                # transfers per 3200-col chunk
                for _ in range(min(n, len(pend))):
                    s, c0, ci, ys = pend.pop(0)
                    for h in range(2):
                        t0 = 128 * ci + 64 * h
                        nc.gpsimd.dma_start(
                            out_d[s, c0:c0 + 128, t0:t0 + 64, :],
                            ys[:, 1600 * h:1600 * h + 1600].rearrange(
                                "p (a b) -> p a b", a=64, b=V),
                            accum_op=ALU.add,
                        )

            ORDER = [0, 2, 4, 6, 1, 3, 5, 7]
            emit_pre(0, make_m1p(0))
            for i, blk in enumerate(ORDER):
                nxt = ORDER[i + 1] if i + 1 < NB else None
                emit_main(blk, nxt)
                if blk == 4:
                    emit_xout(1)
                elif blk == 6:
                    emit_ar_coef(0)
                    emit_ph2_ts(0)
                    emit_xout(3)
                elif blk == 1:
                    flush_accums(4)
                elif blk == 3:
                    flush_accums(2)
                    emit_xout(5)
                    flush_accums(2)
                elif blk == 5:
                    flush_accums(2)
                    emit_xout(7)
                    flush_accums(2)
                elif blk == 7:
                    flush_accums(4)
            emit_ar_coef(1)
            emit_ph2_ts(1)
            flush_accums(len(pend))

            p2s.__exit__(None, None, None)
            p2sm.__exit__(None, None, None)
            for pc in (p1h, p1g, p1f, p1e, p1cc, p1c, p1b):
                pc.__exit__(None, None, None)

    nc.compile()
    return nc


def _host_prep(x, A, Wq, Wk, alpha, Wg, gamma, beta):
    bf = ml_dtypes.bfloat16
    A_sum = A.sum(axis=0)
    A_phys = A_sum / np.clip(A_sum.sum(axis=-1, keepdims=True), 1e-6, None)
    scl = 1.0 / (T * d_k ** 0.25)

    xw = np.zeros((TP, 128, 128), np.float32)
    wqk = np.zeros((TP, 128, 112), np.float32)
    for tp in range(TP):
        for gi in range(2):
            g = 2 * tp + gi
            r = slice(64 * gi, 64 * gi + 64)
            xw[tp][r, r] = Wg[g].T
            wqk[tp][r, 64 * gi:64 * gi + 16] = scl * Wq[g].T
            wqk[tp][r, 64 * gi + 32:64 * gi + 48] = scl * Wk[g].T

    talpha = np.repeat(np.tanh(alpha)[None, :], V, axis=0).astype(np.float32)
    sel = np.zeros((V, 4 * CH), np.float32)
    for d in range(4):
        sel[:, CH * d + V * d:CH * d + V * d + V] = np.eye(V)
    gb4 = np.concatenate(
        [gamma.reshape(TP, 128).T, beta.reshape(TP, 128).T], axis=1
    ).astype(np.float32)
    common = {
        "sel": sel.astype(bf),
        "xw": xw.astype(bf),
        "wqk": wqk.astype(bf),
        "aphys": A_phys.astype(np.float32),
        "talpha": talpha,
        "ident": np.eye(V, dtype=np.float32),
        "gb4": gb4,
    }
    return common


def kernel(x, A, Wq, Wk, alpha, Wg, gamma, beta, _trace=False, _trace_kwargs=None):
    x = np.asarray(x, np.float32)
    common = _host_prep(
        x,
        np.asarray(A, np.float32),
        np.asarray(Wq, np.float32),
        np.asarray(Wk, np.float32),
        np.asarray(alpha, np.float32),
        np.asarray(Wg, np.float32),
        np.asarray(gamma, np.float32),
        np.asarray(beta, np.float32),
    )
    if "nc" not in _CACHE:
        _CACHE["nc"] = _build()
    nc = _CACHE["nc"]

    in_maps = []
    for ci in range(N_CORES):
        m = dict(common)
        m["x"] = np.ascontiguousarray(x[BL * ci:BL * ci + BL])
        in_maps.append(m)

    kw = {}
    if _trace:
        kw = dict(trace=True, trace_kwargs=_trace_kwargs or {})
    res = bass_utils.run_bass_kernel_spmd(
        nc, in_maps, core_ids=list(range(N_CORES)), **kw
    )
    out = np.concatenate([r["out"] for r in res.results], axis=0)
    _CACHE["last_result"] = res
    return out


# revision 43
# speedup vs baseline: 1.5719x; 1.0021x over previous
"""AdaptiveCTRGCN distributed Trainium2 kernel (8 NeuronCores, batch-parallel).

Shapes (hardcoded): x (32,256,256,25) f32, A (3,25,25), Wq/Wk (4,16,64),
alpha (4,), Wg (4,64,64), gamma/beta (256,).
Per core: 4 samples. Two channel-halves ("tilepairs") of 128 channels
(= 2 groups of 64). BatchNorm statistics all-reduced across the 8 cores.

v4: gpsimd casting DMAs move x f32->bf16 straight into resident SBUF tiles.
Phase 1 also copies x -> out via DRAM->DRAM DMA during idle DMA time; the
conv output ob overwrites the input tile in place for every block. Phase 2
computes z = ob*inv+b2 in DVE 4x bf16 mode and DMA-accumulates z into the
output (out += z), so no spill, no re-read, and no residual-add pass.
The adjacency chain of block b+1 is software-pipelined into block b's
matmul loop.
"""
import sys

sys.path.insert(0, "/opt/trn_rl_repo")

import numpy as np
import ml_dtypes
from concourse import bass, bacc, tile, mybir, bass_utils

F32 = mybir.dt.float32
BF16 = mybir.dt.bfloat16
AF = mybir.ActivationFunctionType
ALU = mybir.AluOpType

N_CORES = 8
B, C, T, V = 32, 256, 256, 25
G, C_g, d_k = 4, 64, 16
BL = B // N_CORES          # samples per core = 4
TP = 2                     # channel halves (128 ch each)
NB = 2 * BL                # blocks per core = 8
QC = 16                    # 400-col quad-chunk groups per half (16 t each)
QW = 400                   # cols per quad-chunk (16 t * 25 v)
CH = 100                   # cols per matmul chunk (4 t * 25 v)
PW = 800                   # cols per m1 pair (2 quad-chunks)
TV = T * V                 # 6400
N_GLOBAL = float(B * T * V)   # BN sample count per channel
BN_EPS = 1e-5

# engine for the PSUM->SBUF ob copy per quad-chunk ('a'=ACT, 'd'=DVE)
OB_ENG = ['a', 'd', 'a', 'd', 'a', 'd', 'a', 'd',
          'a', 'd', 'a', 'd', 'a', 'a', 'a', 'a']

_CACHE = {}


def _build(single_core=False):
    nc = bacc.Bacc(
        "TRN2", target_bir_lowering=False, debug=False,
        num_devices=1 if single_core else N_CORES,
    )

    x_d = nc.dram_tensor("x", [BL, C, T, V], F32, kind="ExternalInput").ap()
    xw_d = nc.dram_tensor("xw", [TP, 128, 128], BF16, kind="ExternalInput").ap()
    wqk_d = nc.dram_tensor("wqk", [TP, 128, 112], BF16, kind="ExternalInput").ap()
    aphys_d = nc.dram_tensor("aphys", [V, V], F32, kind="ExternalInput").ap()
    talpha_d = nc.dram_tensor("talpha", [V, G], F32, kind="ExternalInput").ap()
    ident_d = nc.dram_tensor("ident", [V, V], F32, kind="ExternalInput").ap()
    sel_d = nc.dram_tensor("sel", [V, 4 * CH], BF16, kind="ExternalInput").ap()
    gb_d = nc.dram_tensor("gb4", [128, 4], F32, kind="ExternalInput").ap()
    out_d = nc.dram_tensor("out", [BL, C, T, V], F32, kind="ExternalOutput").ap()

    with tile.TileContext(nc) as tc:
        with (
            tc.tile_pool(name="const", bufs=1) as cpool,
            tc.tile_pool(name="dram", bufs=2, space="DRAM") as dpool,
        ):
            # ---- constants ----
            xw_sb = []
            wqk_sb = []
            for tp in range(TP):
                t1 = cpool.tile([128, 128], BF16, tag=f"xw{tp}")
                nc.sync.dma_start(t1[:], xw_d[tp])
                xw_sb.append(t1)
                t2 = cpool.tile([128, 112], BF16, tag=f"wqk{tp}")
                nc.sync.dma_start(t2[:], wqk_d[tp])
                wqk_sb.append(t2)
            gb_sb = cpool.tile([128, 4], F32, tag="gb4")
            nc.sync.dma_start(gb_sb[:], gb_d[:])
            aphys_sb = cpool.tile([V, V], F32, tag="aphys")
            nc.sync.dma_start(aphys_sb[:], aphys_d[:])
            talpha_sb = cpool.tile([V, G], F32, tag="talpha")
            nc.sync.dma_start(talpha_sb[:], talpha_d[:])
            ident_sb = cpool.tile([V, V], F32, tag="ident")
            nc.sync.dma_start(ident_sb[:], ident_d[:])
            sel_sb = cpool.tile([V, 4 * CH], BF16, tag="sel")
            nc.sync.dma_start(sel_sb[:], sel_d[:])

            # persistent per-block tiles (bf16): input copies, overwritten
            # in place by the conv output ob during the block's main loop.
            xb_t = [cpool.tile([128, TV], BF16, tag=f"xb{i}", name=f"xb{i}")
                    for i in range(NB)]
            stat_c = [cpool.tile([128, 2, BL], F32, tag=f"statc{tp}",
                                 name=f"statc{tp}")
                      for tp in range(TP)]
            lg = cpool.tile([128, 8], F32, tag="lg")
            ivb2 = cpool.tile([128, 4], F32, tag="ivb2")

            # ---- phase 1 pools ----
            p1b = tc.tile_pool(name="xwt", bufs=8)
            xwtpool = p1b.__enter__()
            p1c = tc.tile_pool(name="small", bufs=2)
            smpool = p1c.__enter__()
            p1cc = tc.tile_pool(name="bnc", bufs=2)
            bncpool = p1cc.__enter__()
            p1e = tc.tile_pool(name="mm1", bufs=2, space="PSUM")
            mm1pool = p1e.__enter__()
            p1f = tc.tile_pool(name="mm2", bufs=2, space="PSUM")
            mm2pool = p1f.__enter__()
            p1g = tc.tile_pool(name="qkp", bufs=1, space="PSUM")
            qkpool = p1g.__enter__()
            p1h = tc.tile_pool(name="smp", bufs=1, space="PSUM")
            smppool = p1h.__enter__()

            # all input loads up front: casting DMA f32->bf16 on gpsimd
            # (block 0 in 4 chunks so its qk pass starts early)
            for blk in range(NB):
                s, tp = blk // 2, blk % 2
                nch = 4 if blk in (0, 2, 4) else 1
                tpc = T // nch
                for hi in range(nch):
                    nc.gpsimd.dma_start(
                        xb_t[blk][:, TV // nch * hi:TV // nch * (hi + 1)]
                        .rearrange("p (a b) -> p a b", a=tpc, b=V),
                        x_d[s, 128 * tp:128 * tp + 128,
                            tpc * hi:tpc * (hi + 1), :],
                    )
            # residual: out = x, streamed DRAM->DRAM behind the input loads
            # on the same gpsimd queue (phase 2 accumulates ob*inv+b2 on top).
            # Emitted per channel-half so each half's accum stream can start
            # as soon as its own residual copies are done.
            def emit_xout(blk):
                s, tp = blk // 2, blk % 2
                nc.gpsimd.dma_start(
                    out_d[s, 128 * tp:128 * tp + 128, :, :],
                    x_d[s, 128 * tp:128 * tp + 128, :, :],
                )

            for blk in (0, 2, 4, 6):
                emit_xout(blk)

            # per-block pipelined state
            st = {}

            def emit_qk(blk, part):
                # 4 accumulating matmuls per call (part 0..3)
                if part == 0:
                    st.setdefault(blk, {})["qk"] = qkpool.tile(
                        [112, QW], F32, name="qkps")
                qk_ps = st[blk]["qk"]
                tp = blk % 2
                for qi in range(4 * part, 4 * part + 4):
                    nc.tensor.matmul(
                        qk_ps[:],
                        wqk_sb[tp][:],
                        xb_t[blk][:, QW * qi:QW * qi + QW],
                        start=(qi == 0),
                        stop=(qi == QC - 1),
                    )

            def emit_qkred(blk):
                qk_ps = st[blk]["qk"]
                for gi in range(2):
                    qt = smpool.tile([16, V], F32, tag=f"qg{gi}")
                    nc.vector.tensor_reduce(
                        qt[:],
                        qk_ps[64 * gi:64 * gi + 16, :].rearrange(
                            "p (t v) -> p v t", t=16, v=V
                        ),
                        axis=mybir.AxisListType.X,
                        op=ALU.add,
                    )
                    st[blk][f"qg{gi}"] = qt
                    kt = smpool.tile([16, V], F32, tag=f"kg{gi}")
                    nc.vector.tensor_reduce(
                        kt[:],
                        qk_ps[64 * gi + 32:64 * gi + 48, :].rearrange(
                            "p (t v) -> p v t", t=16, v=V
                        ),
                        axis=mybir.AxisListType.X,
                        op=ALU.add,
                    )
                    st[blk][f"kg{gi}"] = kt

            def emit_sc(blk, gi):
                # scores matmul + softmax chain -> ag (f32 [V,V])
                tp = blk % 2
                g = 2 * tp + gi
                sc_ps = smppool.tile([V, V], F32, tag="smt", name="scps")
                nc.tensor.matmul(
                    sc_ps[:],
                    st[blk][f"qg{gi}"][:],
                    st[blk][f"kg{gi}"][:],
                    start=True, stop=True,
                )
                mx = smpool.tile([V, 1], F32, tag=f"mx{gi}")
                nc.vector.tensor_reduce(
                    mx[:], sc_ps[:], axis=mybir.AxisListType.X,
                    op=ALU.max, negate=True,
                )
                nc.scalar.activation(sc_ps[:], sc_ps[:], AF.Exp, bias=mx[:])
                smrr = smpool.tile([V, 3], F32, tag=f"smrr{gi}")
                nc.vector.tensor_reduce(
                    smrr[:, 0:1], sc_ps[:], axis=mybir.AxisListType.X,
                    op=ALU.add
                )
                nc.vector.reciprocal(smrr[:, 1:2], smrr[:, 0:1])
                rst = smrr[:, 2:3]
                nc.vector.tensor_scalar_mul(
                    rst, smrr[:, 1:2], talpha_sb[:, g:g + 1]
                )
                ag = smpool.tile([V, V], F32, tag=f"ag{gi}")
                nc.vector.scalar_tensor_tensor(
                    ag[:], sc_ps[:], rst, aphys_sb[:],
                    op0=ALU.mult, op1=ALU.add,
                )
                st[blk][f"ag{gi}"] = ag

            def emit_tr(blk, gi):
                agt_ps = smppool.tile([V, V], F32, tag="smt", name="agtps")
                nc.tensor.transpose(agt_ps[:], st[blk][f"ag{gi}"][:], ident_sb[:])
                agtb = smpool.tile([V, V], BF16, tag=f"agtb{gi}")
                nc.vector.tensor_copy(agtb[:], agt_ps[:])
                st[blk][f"agtb{gi}"] = agtb

            def emit_i4(blk, gi):
                i4a_ps = smppool.tile([CH, CH], F32, tag="smt", name="i4aps")
                agtb = st[blk][f"agtb{gi}"]
                for d in range(4):
                    nc.tensor.matmul(
                        i4a_ps[:, V * d:V * d + V],
                        sel_sb[:, CH * d:CH * d + CH],
                        agtb[:],
                        start=True, stop=True,
                    )
                i4a = smpool.tile([CH, CH], BF16, tag=f"i4a{gi}")
                nc.scalar.activation(i4a[:], i4a_ps[:], AF.Copy)
                st[blk][f"i4a{gi}"] = i4a

            def emit_pre(blk, m1p=None):
                # standalone prologue for block 0: interleave its own m1
                # pairs so the PE has work while the softmax chain runs
                for part in range(4):
                    emit_qk(blk, part)
                emit_qkred(blk)
                steps = [
                    ("sc", 0), ("m1", 0), ("sc", 1), ("m1", 1),
                    ("tr", 0), ("m1", 2), ("tr", 1), ("m1", 3),
                    ("i4", 0), ("m1", 4), ("i4", 1), ("m1", 5),
                ]
                for kind, arg in steps:
                    if kind == "sc":
                        emit_sc(blk, arg)
                    elif kind == "tr":
                        emit_tr(blk, arg)
                    elif kind == "i4":
                        emit_i4(blk, arg)
                    elif m1p is not None:
                        m1p(arg)

            def make_m1p(blk):
                tp = blk % 2
                xb = xb_t[blk]
                xwt_p = st.setdefault(blk, {}).setdefault("xwt", {})

                def do_m1p(pi):
                    if pi in xwt_p:
                        return
                    mp = mm1pool.tile([CH, 1024], F32, name="mp")
                    for j in range(8):
                        nc.tensor.matmul(
                            mp[:, 128 * j:128 * j + 128],
                            xb[:, PW * pi + CH * j:PW * pi + CH * j + CH],
                            xw_sb[tp][:],
                            start=True, stop=True,
                        )
                    xwt = xwtpool.tile([CH, 1024], BF16, tag="xwt", name="xwt")
                    nc.scalar.activation(xwt[:], mp[:], AF.Copy)
                    xwt_p[pi] = xwt

                return do_m1p

            def emit_main(blk, nxt):
                """m1/m2/copies/bn for blk; interleaves PRE of nxt."""
                tp = blk % 2
                xb = xb_t[blk]
                ob = xb_t[blk]
                do_m1p = make_m1p(blk)
                xwt_p = st[blk]["xwt"]

                do_m1p(0)
                do_m1p(1)
                bnc = bncpool.tile([128, QC, 6], F32, tag="bnc")
                i4a = [st[blk]["i4a0"], st[blk]["i4a1"]]
                for qi in range(QC):
                    pi = qi // 2
                    if qi % 2 == 0 and pi + 2 <= 7:
                        do_m1p(pi + 2)
                    xwt = xwt_p[pi]
                    base = 512 * (qi % 2)
                    op = mm2pool.tile([128, QW], F32)
                    for h in range(4):
                        for gi in range(2):
                            nc.tensor.matmul(
                                op[64 * gi:64 * gi + 64, CH * h:CH * h + CH],
                                xwt[:, base + 128 * h + 64 * gi:
                                    base + 128 * h + 64 * gi + 64],
                                i4a[gi][:],
                                start=True, stop=True,
                            )
                    dst = ob[:, QW * qi:QW * qi + QW]
                    if OB_ENG[qi] == 'd':
                        nc.vector.tensor_copy(dst, op[:])
                    else:
                        nc.scalar.activation(dst, op[:], AF.Copy)
                    lag = 0 if blk == NB - 1 else 3
                    if qi >= lag:
                        qj = qi - lag
                        nc.vector.bn_stats(
                            bnc[:, qj, :], ob[:, QW * qj:QW * qj + QW]
                        )
                    # interleave next block's qk/adjacency chain
                    if nxt is not None:
                        if qi < 4:
                            emit_qk(nxt, qi)
                        elif qi == 4:
                            emit_qkred(nxt)
                        elif qi == 5:
                            emit_sc(nxt, 0)
                        elif qi == 6:
                            emit_sc(nxt, 1)
                        elif qi == 9:
                            emit_tr(nxt, 0)
                        elif qi == 10:
                            emit_tr(nxt, 1)
                        elif qi == 12:
                            emit_i4(nxt, 0)
                        elif qi == 13:
                            emit_i4(nxt, 1)
                if blk != NB - 1:
                    for qj in (QC - 2, QC - 1):
                        nc.vector.bn_stats(
                            bnc[:, qj, :], ob[:, QW * qj:QW * qj + QW]
                        )

                # block stats -> sum/ssq columns
                s = blk // 2
                msv = smpool.tile([128, 4], F32, tag="msv")
                nc.vector.bn_aggr(
                    msv[:, 0:2], bnc[:].rearrange("p a b -> p (a b)")
                )
                nc.vector.tensor_scalar_mul(
                    stat_c[tp][:, 0, s:s + 1], msv[:, 0:1], float(TV)
                )
                m2c = msv[:, 2:3]
                nc.vector.tensor_tensor(
                    m2c, msv[:, 0:1], msv[:, 0:1], op=ALU.mult
                )
                nc.vector.tensor_tensor(m2c, m2c, msv[:, 1:2], op=ALU.add)
                nc.vector.tensor_scalar_mul(
                    stat_c[tp][:, 1, s:s + 1], m2c, float(TV)
                )

            # per-half all-reduce stand-in + BN coefficients.
            # lg layout: cols 2*tp..2*tp+1 local (sum, ssq); 4+2*tp global.
            p2sm = tc.tile_pool(name="small2", bufs=1)
            smpool2 = p2sm.__enter__()
            p2s = tc.tile_pool(name="ys", bufs=8)
            yspool = p2s.__enter__()
            inv_sb = [ivb2[:, tp:tp + 1] for tp in range(TP)]
            b2_sb = [ivb2[:, 2 + tp:3 + tp] for tp in range(TP)]

            def emit_ar_coef(tp):
                loc = lg[:, 2 * tp:2 * tp + 2]
                glob = lg[:, 4 + 2 * tp:6 + 2 * tp]
                red_dst = glob if single_core else loc
                for j in range(2):
                    nc.vector.tensor_reduce(
                        red_dst[:, j:j + 1],
                        stat_c[tp][:, j, :],
                        axis=mybir.AxisListType.X,
                        op=ALU.add,
                    )
                if single_core:
                    pass
                else:
                    cin = dpool.tile([128, 2], F32, name=f"cin{tp}")
                    cout = dpool.tile([128, 2], F32, name=f"cout{tp}")
                    nc.sync.dma_start(cin[:], loc)
                    nc.gpsimd.collective_compute(
                        "AllReduce",
                        ALU.add,
                        replica_groups=[list(range(N_CORES))],
                        ins=[cin[:].opt()],
                        outs=[cout[:].opt()],
                    )
                    nc.sync.dma_start(glob, cout[:])
                # inv = gamma/sqrt(var+eps); b2 = beta - mu*inv
                scr = smpool2.tile([128, 4], F32, tag=f"scr{tp}")
                mu = scr[:, 0:1]
                ex2 = scr[:, 1:2]
                nc.vector.tensor_scalar_mul(scr[:, 0:2], glob, 1.0 / N_GLOBAL)
                musq = scr[:, 2:3]
                nc.vector.tensor_tensor(musq, mu, mu, op=ALU.mult)
                var = scr[:, 1:2]
                nc.vector.tensor_tensor(var, ex2, musq, op=ALU.subtract)
                nc.vector.tensor_scalar_add(var, var, BN_EPS)
                sq = scr[:, 2:3]
                nc.scalar.activation(sq, var, AF.Sqrt)
                rs = scr[:, 3:4]
                nc.vector.reciprocal(rs, sq)
                iv = ivb2[:, tp:tp + 1]
                nc.vector.tensor_tensor(
                    iv, rs, gb_sb[:, tp:tp + 1], op=ALU.mult)
                mi = scr[:, 2:3]
                nc.vector.tensor_tensor(mi, mu, iv, op=ALU.mult)
                nc.vector.tensor_tensor(
                    ivb2[:, 2 + tp:3 + tp], gb_sb[:, 2 + tp:3 + tp], mi,
                    op=ALU.subtract)

            pend = []

            def emit_ph2_ts(tp):
                # out += ob*inv + b2: DVE 4x pass now, accum DMAs deferred
                # so later compute isn't scheduled behind the whole stream
                first = True
                for blk in range(tp, NB, 2):
                    s = blk // 2
                    ob = xb_t[blk]
                    c0 = 128 * tp
                    for ci in range(2):
                        cols = slice(3200 * ci, 3200 * ci + 3200)
                        ys = yspool.tile([128, 3200], BF16, tag="ys")
                        if first:
                            # split the first chunk so its first 1600-col
                            # accum can fire before the whole ts finishes
                            for h2 in range(2):
                                nc.vector.tensor_scalar(
                                    ys[:, 1600 * h2:1600 * h2 + 1600],
                                    ob[:, 3200 * ci + 1600 * h2:
                                       3200 * ci + 1600 * h2 + 1600],
                                    inv_sb[tp], b2_sb[tp],
                                    op0=ALU.mult, op1=ALU.add,
                                )
                            first = False
                        else:
                            nc.vector.tensor_scalar(
                                ys[:], ob[:, cols], inv_sb[tp], b2_sb[tp],
                                op0=ALU.mult, op1=ALU.add,
                            )
                        pend.append((s, c0, ci, ys))

            def flush_accums(n):
                # accum DMAs >1600 cols corrupt (hw-verified): two 1600-col
                # transfers per 3200-col chunk
                for _ in range(min(n, len(pend))):
                    s, c0, ci, ys = pend.pop(0)
                    for h in range(2):
                        t0 = 128 * ci + 64 * h
                        nc.gpsimd.dma_start(
                            out_d[s, c0:c0 + 128, t0:t0 + 64, :],
                            ys[:, 1600 * h:1600 * h + 1600].rearrange(
                                "p (a b) -> p a b", a=64, b=V),
                            accum_op=ALU.add,
                        )

            ORDER = [0, 2, 4, 6, 1, 3, 5, 7]
            emit_pre(0, make_m1p(0))
            for i, blk in enumerate(ORDER):
                nxt = ORDER[i + 1] if i + 1 < NB else None
                emit_main(blk, nxt)
                if blk == 4:
                    emit_xout(1)
                elif blk == 6:
                    emit_ar_coef(0)
                    emit_ph2_ts(0)
                    emit_xout(3)
                elif blk == 1:
                    flush_accums(4)
                elif blk == 3:
                    flush_accums(2)
                    emit_xout(5)
                    flush_accums(2)
                elif blk == 5:
                    flush_accums(2)
                    emit_xout(7)
                    flush_accums(2)
                elif blk == 7:
                    flush_accums(4)
            emit_ar_coef(1)
            emit_ph2_ts(1)
            flush_accums(len(pend))

            p2s.__exit__(None, None, None)
            p2sm.__exit__(None, None, None)
            for pc in (p1h, p1g, p1f, p1e, p1cc, p1c, p1b):
                pc.__exit__(None, None, None)

    nc.compile()
    return nc


def _host_prep(x, A, Wq, Wk, alpha, Wg, gamma, beta):
    bf = ml_dtypes.bfloat16
    A_sum = A.sum(axis=0)
    A_phys = A_sum / np.clip(A_sum.sum(axis=-1, keepdims=True), 1e-6, None)
    scl = 1.0 / (T * d_k ** 0.25)

    xw = np.zeros((TP, 128, 128), np.float32)
    wqk = np.zeros((TP, 128, 112), np.float32)
    for tp in range(TP):
        for gi in range(2):
            g = 2 * tp + gi
            r = slice(64 * gi, 64 * gi + 64)
            xw[tp][r, r] = Wg[g].T
            wqk[tp][r, 64 * gi:64 * gi + 16] = scl * Wq[g].T
            wqk[tp][r, 64 * gi + 32:64 * gi + 48] = scl * Wk[g].T

    talpha = np.repeat(np.tanh(alpha)[None, :], V, axis=0).astype(np.float32)
    sel = np.zeros((V, 4 * CH), np.float32)
    for d in range(4):
        sel[:, CH * d + V * d:CH * d + V * d + V] = np.eye(V)
    gb4 = np.concatenate(
        [gamma.reshape(TP, 128).T, beta.reshape(TP, 128).T], axis=1
    ).astype(np.float32)
    common = {
        "sel": sel.astype(bf),
        "xw": xw.astype(bf),
        "wqk": wqk.astype(bf),
        "aphys": A_phys.astype(np.float32),
        "talpha": talpha,
        "ident": np.eye(V, dtype=np.float32),
        "gb4": gb4,
    }
    return common


def kernel(x, A, Wq, Wk, alpha, Wg, gamma, beta, _trace=False, _trace_kwargs=None):
    x = np.asarray(x, np.float32)
    common = _host_prep(
        x,
        np.asarray(A, np.float32),
        np.asarray(Wq, np.float32),
        np.asarray(Wk, np.float32),
        np.asarray(alpha, np.float32),
        np.asarray(Wg, np.float32),
        np.asarray(gamma, np.float32),
        np.asarray(beta, np.float32),
    )
    if "nc" not in _CACHE:
        _CACHE["nc"] = _build()
    nc = _CACHE["nc"]

    in_maps = []
    for ci in range(N_CORES):
        m = dict(common)
        m["x"] = np.ascontiguousarray(x[BL * ci:BL * ci + BL])
        in_maps.append(m)

    kw = {}
    if _trace:
        kw = dict(trace=True, trace_kwargs=_trace_kwargs or {})
    res = bass_utils.run_bass_kernel_spmd(
        nc, in_maps, core_ids=list(range(N_CORES)), **kw
    )
    out = np.concatenate([r["out"] for r in res.results], axis=0)
    _CACHE["last_result"] = res
    return out
